# revision 1
# baseline (speedup 1.0000x reference)
"""GCN (3x GCNConv + global mean pool + linear) on 8 Trainium2 NeuronCores.

Strategy (dst-sharded message passing):
  - Nodes are sharded n/8 per core; each core's nodes are permuted into
    windows of 32 (degree-balanced) -> supertiles of 128 (PSUM tiles).
  - Edges are partitioned by dst core and packed into (window, class) tiles
    of 128 edges; class = which signed-int16-indexable half of the node
    table the src row lives in (dma_gather indices are int16).
  - Per layer: dma_gather pulls 256B fp16 rows of the scaled node table
    h_hat = dinv*h from HBM; TensorE computes the scatter-add as S^T @ M
    matmuls (S = [128e, 32d] fp16 of edge weights, built on DVE via
    iota==dstslot compare); PSUM accumulates per window.
  - Norm + self loop folded algebraically:
        gcn_conv(h) = (dinv * (A_ew @ h_hat + h_hat)) @ W + b,  h_hat = dinv*h
  - Per supertile: u = agg*dinv + hown2 (hown2 = h*dinv^2 resident in SBUF),
    PE transpose, f32 GEMM with W, bias(+relu) on ACT, transpose back,
    rescale + fp16 table write. Tables distributed with AllGather.
  - Pooling: matmul with host-built P (1[batch==g]) accumulated over
    supertiles -> AllReduce -> final linear on-device -> out [64, 5] f32.
"""

import os
import sys
import numpy as np

for _p in ("/opt/trn_rl_repo", "/root/.axon_site/_ro/trn_rl_repo"):
    if os.path.isdir(_p) and _p not in sys.path:
        sys.path.insert(0, _p)

N_CORES = 8
N_GRAPHS = 64
HID = 128
N_CLASS = 5
F_IN = 7
F_PAD = 8
WIN = 32
SUP = 128
GROUP_SUPS = 3
CHUNK_MAX = 32
GATH_BUFS = 4
SINGLE_PACKET = False  # True requires chunks of <= 8 tiles (1024 descriptors)
IDX_CAP = 32768


def _group_ranks(keys, n_keys):
    """rank of each element within its key group (keys int array)."""
    nk = len(keys)
    if nk == 0:
        return np.zeros(0, dtype=np.int64)
    order = np.argsort(keys, kind="stable")
    sk = keys[order]
    is_new = np.r_[True, sk[1:] != sk[:-1]]
    gs_idx = np.nonzero(is_new)[0]
    gs = np.repeat(gs_idx, np.diff(np.r_[gs_idx, nk]))
    rank = np.empty(nk, dtype=np.int64)
    rank[order] = np.arange(nk) - gs
    return rank


class Plan:
    pass


def build_plan(x, edge_index, edge_attr, batch, n_cores=N_CORES, n_graphs=N_GRAPHS):
    """Host-side sharding/layout planning: pure permutation / zero-padding of
    inputs, no arithmetic on float data."""
    p = Plan()
    n = x.shape[0]
    assert n % n_cores == 0
    npc = n // n_cores
    nsup = (npc + SUP - 1) // SUP
    padc = nsup * SUP
    npad = n_cores * padc
    nwin = padc // WIN
    p.n, p.npc, p.nsup, p.padc, p.npad, p.nwin = n, npc, nsup, padc, npad, nwin
    p.n_cores, p.n_graphs = n_cores, n_graphs
    base_hi = max(0, npad - IDX_CAP)
    p.base_hi = base_hi

    src = np.asarray(edge_index[0], dtype=np.int64)
    dst = np.asarray(edge_index[1], dtype=np.int64)
    ew = np.asarray(edge_attr, dtype=np.float32)
    batch = np.asarray(batch, dtype=np.int64)

    # ---- window assignment (degree-balanced snake over sorted degrees) ----
    indeg = np.bincount(dst, minlength=n)
    prow = np.empty(n, dtype=np.int64)
    win_all = np.empty(n, dtype=np.int64)
    slot_all = np.empty(n, dtype=np.int64)
    for c in range(n_cores):
        lo = c * npc
        order = np.argsort(-indeg[lo : lo + npc], kind="stable")
        pos = np.empty(npc, dtype=np.int64)
        pos[order] = np.arange(npc)
        rnd = pos // nwin
        off = pos % nwin
        w = np.where(rnd % 2 == 0, off, nwin - 1 - off)
        plid = (w // 4) * SUP + (w % 4) * WIN + rnd
        prow[lo : lo + npc] = c * padc + plid
        win_all[lo : lo + npc] = w
        slot_all[lo : lo + npc] = rnd
    p.prow = prow

    # ---- per-core-window tile budgets (uniform across cores) ----
    ecore = dst // npc
    esrc_prow = prow[src]
    can_lo = esrc_prow < IDX_CAP
    can_hi = esrc_prow >= base_hi
    ewin = win_all[dst]

    flo_cw = np.zeros((n_cores, nwin), dtype=np.int64)
    fhi_cw = np.zeros((n_cores, nwin), dtype=np.int64)
    tot_cw = np.zeros((n_cores, nwin), dtype=np.int64)
    np.add.at(tot_cw, (ecore, ewin), 1)
    np.add.at(flo_cw, (ecore[~can_hi], ewin[~can_hi]), 1)
    np.add.at(fhi_cw, (ecore[~can_lo], ewin[~can_lo]), 1)

    t_lo = np.max((flo_cw + 127) // 128, axis=0)
    t_hi = np.max((fhi_cw + 127) // 128, axis=0)
    grow = np.maximum(np.max(tot_cw, axis=0) - (t_lo + t_hi) * 128, 0)
    t_lo = t_lo + (grow + 127) // 128
    t_lo = np.maximum(t_lo, (t_lo + t_hi) == 0)
    p.t_lo, p.t_hi = t_lo, t_hi

    # ---- global tile order: groups of supertiles, class runs within group --
    n_groups = (nsup + GROUP_SUPS - 1) // GROUP_SUPS
    p.n_groups = n_groups
    tiles = []  # (win, cls)
    chunks = []  # (tile_start, ntiles, cls)
    groups = []  # (sup_start, nsups, [chunk idx], (t0, t1))
    for g in range(n_groups):
        s0 = g * GROUP_SUPS
        ns = min(GROUP_SUPS, nsup - s0)
        wlist = range(s0 * 4, (s0 + ns) * 4)
        g_t0 = len(tiles)
        g_chunks = []
        for cls in (0, 1):
            run_t0 = len(tiles)
            for w in wlist:
                tc = int(t_lo[w]) if cls == 0 else int(t_hi[w])
                tiles.extend((w, cls) for _ in range(tc))
            nrun = len(tiles) - run_t0
            t0 = run_t0
            while nrun > 0:
                take = min(CHUNK_MAX, nrun)
                g_chunks.append(len(chunks))
                chunks.append((t0, take, cls))
                t0 += take
                nrun -= take
        groups.append((s0, ns, g_chunks, (g_t0, len(tiles))))
    p.tiles, p.chunks, p.groups = tiles, chunks, groups
    p.tot = len(tiles)
    p.chunk_of = np.zeros(p.tot, dtype=np.int64)
    for ci, (t0, ntl, _c) in enumerate(chunks):
        p.chunk_of[t0 : t0 + ntl] = ci

    # first/last tile of each window (for PSUM start/stop flags) and the
    # per-window tile list (matmuls are emitted window-major so that PSUM
    # has_written accumulation groups never interleave within a bank)
    tw = np.array([t[0] for t in tiles])
    p.first_of_win = np.zeros(p.tot, dtype=bool)
    p.last_of_win = np.zeros(p.tot, dtype=bool)
    p.win_tiles = {}
    for w in range(nwin):
        ids = np.nonzero(tw == w)[0]
        p.first_of_win[ids.min()] = True
        p.last_of_win[ids.max()] = True
        p.win_tiles[w] = [int(i) for i in ids]
    # first tile index of each (win, cls) run
    t_off = {}
    for t, (w, cls) in enumerate(tiles):
        t_off.setdefault((w, cls), t)

    counts = np.bincount(batch, minlength=n_graphs).astype(np.float32)
    p.counts = counts
    L = int(indeg.max()) + 1
    p.L = L

    # ---- per-core arrays ----
    p.per_core = []
    for c in range(n_cores):
        m = ecore == c
        ed, ee = dst[m], ew[m]
        eprow = esrc_prow[m]
        ewin_c = ewin[m]
        eslot = slot_all[ed]
        e_can_hi = can_hi[m]
        e_can_lo = can_lo[m]
        ne = len(ed)

        # per-edge class: fill lo up to its target, rest hi
        ecls = np.full(ne, -1, dtype=np.int64)
        ecls[~e_can_hi] = 0
        ecls[~e_can_lo] = 1
        free = ecls == -1
        # per-window lo target
        tot_w = np.bincount(ewin_c, minlength=nwin)
        flo_w = np.bincount(ewin_c[~e_can_hi], minlength=nwin)
        lo_target = np.maximum(flo_w, tot_w - t_hi * 128)
        lo_target = np.minimum(lo_target, t_lo * 128)
        # rank of free edges within window
        fidx = np.nonzero(free)[0]
        frank = _group_ranks(ewin_c[fidx], nwin)
        to_lo = frank < (lo_target - flo_w)[ewin_c[fidx]]
        ecls[fidx[to_lo]] = 0
        ecls[fidx[~to_lo]] = 1

        # slot position within (win, cls) run
        key = ewin_c * 2 + ecls
        k = _group_ranks(key, nwin * 2)
        t_off_arr = np.zeros((nwin, 2), dtype=np.int64)
        for (wv, cv), tv in t_off.items():
            t_off_arr[wv, cv] = tv
        run0 = t_off_arr[ewin_c, ecls]
        t_of_e = run0 + k // 128
        p_of_e = k % 128

        idx_arr = np.zeros((p.tot, 128), dtype=np.int16)
        slot_arr = np.zeros((p.tot, 128), dtype=np.float16)
        ew_arr = np.zeros((p.tot, 128), dtype=np.float16)
        rel = eprow - np.where(ecls == 1, base_hi, 0)
        assert rel.min() >= 0 and rel.max() < IDX_CAP
        idx_arr[t_of_e, p_of_e] = rel.astype(np.int16)
        slot_arr[t_of_e, p_of_e] = eslot.astype(np.float16)
        ew_arr[t_of_e, p_of_e] = ee.astype(np.float16)

        # wrapped idx layout [16, tot*8], replicated to [128, tot*8]
        idx16 = np.zeros((16, p.tot * 8), dtype=np.int16)
        for ppart in range(128):
            idx16[ppart % 16, np.arange(p.tot) * 8 + ppart // 16] = idx_arr[:, ppart]
        idx128 = np.ascontiguousarray(np.tile(idx16, (8, 1)))

        dstslot = np.ascontiguousarray(slot_arr.T)  # [128, tot] fp16
        ews = np.ascontiguousarray(ew_arr.T)  # [128, tot] fp16

        # deg accumulation layout [128, nsup*L] (plid -> [plid%128, plid//128 * L + k])
        ewp = np.zeros((128, nsup * L), dtype=np.float32)
        plid_own = (prow[ed] - c * padc).astype(np.int64)
        kk = _group_ranks(plid_own, padc)
        ewp[plid_own % 128, (plid_own // 128) * L + kk] = ee
        p.per_core.append(dict(idx128=idx128, dstslot=dstslot, ews=ews, ewp=ewp))

    # ---- node-indexed arrays ----
    # xpad [npad, F_PAD] f32 (global, by prow) — same for every core
    xf = np.asarray(x, dtype=np.float32)
    xpad = np.zeros((npad, F_PAD), dtype=np.float32)
    xpad[prow, :F_IN] = xf
    p.xpad = xpad
    # per-core x_own [padc, F_PAD]
    p.x_own = []
    p.pmat = []
    for c in range(n_cores):
        lo = c * npc
        xo = np.zeros((padc, F_PAD), dtype=np.float32)
        plid = prow[lo : lo + npc] - c * padc
        xo[plid, :F_IN] = xf[lo : lo + npc]
        p.x_own.append(xo)
        # pool matrix [128, nsup*64] fp16: 1.0 at [plid%128, (plid//128)*G + batch]
        pm = np.zeros((128, nsup * n_graphs), dtype=np.float16)
        pm[plid % 128, (plid // 128) * n_graphs + batch[lo : lo + npc]] = 1.0
        p.pmat.append(np.ascontiguousarray(pm))

    # iota const [128, CHUNK_MAX*WIN] fp16: value d at col t*WIN+d
    p.iota = np.ascontiguousarray(
        np.broadcast_to(
            np.tile(np.arange(WIN, dtype=np.float16), CHUNK_MAX), (128, CHUNK_MAX * WIN)
        )
    )
    p.identity = np.eye(128, dtype=np.float32)
    return p


def build_weight_arrays(p, W1, b1, W2, b2, W3, b3, Wl, bl):
    """Zero-pad / reshape weights (no arithmetic)."""
    w1p = np.zeros((F_PAD, HID), dtype=np.float32)
    w1p[:F_IN] = np.asarray(W1, dtype=np.float32)
    a = dict(
        w1=w1p,
        w2=np.asarray(W2, dtype=np.float32),
        w3=np.asarray(W3, dtype=np.float32),
        wl=np.asarray(Wl, dtype=np.float32),
        b1=np.asarray(b1, dtype=np.float32).reshape(HID, 1),
        b2=np.asarray(b2, dtype=np.float32).reshape(HID, 1),
        b3=np.asarray(b3, dtype=np.float32).reshape(HID, 1),
        blrep=np.ascontiguousarray(
            np.broadcast_to(np.asarray(bl, dtype=np.float32), (p.n_graphs, N_CLASS))
        ),
        invc=(1.0 / np.maximum(p.counts, 1.0)).reshape(p.n_graphs, 1),
    )
    return a


# ----------------------------------------------------------------------------
# Device program
# ----------------------------------------------------------------------------
def build_program(p, enable_asserts=False):
    import dataclasses
    import concourse.bass as bass
    import concourse.bacc as bacc
    import concourse.tile as tile
    import concourse.mybir as mybir

    dt = mybir.dt
    f32, f16, i16 = dt.float32, dt.float16, dt.int16
    Alu = mybir.AluOpType
    Act = mybir.ActivationFunctionType
    G = p.n_graphs
    rg = [list(range(p.n_cores))]

    def bc(ap, nrep):
        """append a step-0 (broadcast) innermost free dim to an AP"""
        return dataclasses.replace(ap, ap=list(ap.ap) + [[0, nrep]])

    nc = bacc.Bacc(
        "TRN2",
        target_bir_lowering=False,
        debug=False,
        enable_asserts=enable_asserts,
        num_devices=p.n_cores,
    )

    # ---- DRAM tensors ----
    xpad_d = nc.dram_tensor("xpad", [p.npad, F_PAD], f32, kind="ExternalInput")
    xown_d = nc.dram_tensor("x_own", [p.padc, F_PAD], f32, kind="ExternalInput")
    ewp_d = nc.dram_tensor("ewp", [128, p.nsup, p.L], f32, kind="ExternalInput")
    idx_d = nc.dram_tensor("idx", [128, p.tot * 8], i16, kind="ExternalInput")
    dstslot_d = nc.dram_tensor("dstslot", [128, p.tot], f16, kind="ExternalInput")
    ews_d = nc.dram_tensor("ews", [128, p.tot], f16, kind="ExternalInput")
    iota_d = nc.dram_tensor("iota", [128, CHUNK_MAX, WIN], f16, kind="ExternalInput")
    pmat_d = nc.dram_tensor("pmat", [128, p.nsup * G], f16, kind="ExternalInput")
    w1_d = nc.dram_tensor("w1", [F_PAD, HID], f32, kind="ExternalInput")
    w2_d = nc.dram_tensor("w2", [HID, HID], f32, kind="ExternalInput")
    w3_d = nc.dram_tensor("w3", [HID, HID], f32, kind="ExternalInput")
    wl_d = nc.dram_tensor("wl", [HID, N_CLASS], f32, kind="ExternalInput")
    b1_d = nc.dram_tensor("b1", [HID, 1], f32, kind="ExternalInput")
    b2_d = nc.dram_tensor("b2", [HID, 1], f32, kind="ExternalInput")
    b3_d = nc.dram_tensor("b3", [HID, 1], f32, kind="ExternalInput")
    invc_d = nc.dram_tensor("invc", [G, 1], f32, kind="ExternalInput")
    blrep_d = nc.dram_tensor("blrep", [G, N_CLASS], f32, kind="ExternalInput")
    ident_d = nc.dram_tensor("ident", [128, 128], f32, kind="ExternalInput")
    out_d = nc.dram_tensor("out", [G, N_CLASS], f32, kind="ExternalOutput")

    table1_d = nc.dram_tensor("table1", [p.npad, HID], f16, kind="Internal")
    agin_d = nc.dram_tensor("agin", [p.padc, HID], f16, kind="Internal")
    table2_d = nc.dram_tensor(
        "table2", [p.npad, HID], f16, kind="Internal", addr_space="Shared"
    )
    table3_d = nc.dram_tensor(
        "table3", [p.npad, HID], f16, kind="Internal", addr_space="Shared"
    )
    dvin_d = nc.dram_tensor("dvin", [p.padc], f32, kind="Internal")
    dvfull_d = nc.dram_tensor(
        "dvfull", [p.npad], f32, kind="Internal", addr_space="Shared"
    )
    arin_d = nc.dram_tensor("arin", [128, G], f32, kind="Internal")
    arout_d = nc.dram_tensor(
        "arout", [128, G], f32, kind="Internal", addr_space="Shared"
    )

    ncols = p.npad // 128  # node-table columns in [128, ncols] layout

    with tile.TileContext(nc) as tc:
        with (
            tc.tile_pool(name="const", bufs=1) as cpool,
            tc.tile_pool(name="gath", bufs=GATH_BUFS) as gpool,
            tc.tile_pool(name="sbld", bufs=GATH_BUFS) as spool,
            tc.tile_pool(name="stage", bufs=3) as stpool,
            tc.tile_pool(name="psagg", bufs=GROUP_SUPS + 1, space="PSUM") as psagg,
            tc.tile_pool(name="psstg", bufs=2, space="PSUM") as psstg,
            tc.tile_pool(name="psacc", bufs=1, space="PSUM") as psacc,
        ):
            # ---- persistent SBUF tiles ----
            ident = cpool.tile([128, 128], f32, tag="ident")
            nc.sync.dma_start(ident[:, :], ident_d[:, :])
            w1 = cpool.tile([F_PAD, HID], f32, tag="w1")
            nc.sync.dma_start(w1[:, :], w1_d[:, :])
            w2 = cpool.tile([HID, HID], f32, tag="w2")
            nc.sync.dma_start(w2[:, :], w2_d[:, :])
            w3 = cpool.tile([HID, HID], f32, tag="w3")
            nc.sync.dma_start(w3[:, :], w3_d[:, :])
            wl = cpool.tile([HID, N_CLASS], f32, tag="wl")
            nc.sync.dma_start(wl[:, :], wl_d[:, :])
            b1 = cpool.tile([HID, 1], f32, tag="b1")
            nc.sync.dma_start(b1[:, :], b1_d[:, :])
            b2 = cpool.tile([HID, 1], f32, tag="b2")
            nc.sync.dma_start(b2[:, :], b2_d[:, :])
            b3 = cpool.tile([HID, 1], f32, tag="b3")
            nc.sync.dma_start(b3[:, :], b3_d[:, :])
            invc = cpool.tile([G, 1], f32, tag="invc")
            nc.sync.dma_start(invc[:, :], invc_d[:, :])
            blrep = cpool.tile([G, N_CLASS], f32, tag="blrep")
            nc.sync.dma_start(blrep[:, :], blrep_d[:, :])
            iota = cpool.tile([128, CHUNK_MAX, WIN], f16, tag="iota")
            nc.sync.dma_start(iota[:, :, :], iota_d[:, :, :])
            pmat = cpool.tile([128, p.nsup * G], f16, tag="pmat")
            nc.sync.dma_start(pmat[:, :], pmat_d[:, :])
            idx_sb = cpool.tile([128, p.tot * 8], i16, tag="idx")
            nc.sync.dma_start(idx_sb[:, :], idx_d[:, :])
            dstslot = cpool.tile([128, p.tot], f16, tag="dstslot")
            nc.sync.dma_start(dstslot[:, :], dstslot_d[:, :])
            ews = cpool.tile([128, p.tot], f16, tag="ews")
            nc.sync.dma_start(ews[:, :], ews_d[:, :])
            dinv_own = cpool.tile([128, p.nsup], f32, tag="dinv_own")
            x2own = cpool.tile([128, p.nsup, F_PAD], f32, tag="x2own")
            hown2a = cpool.tile([128, p.nsup * HID], f32, tag="hown2a")
            hown2b = cpool.tile([128, p.nsup * HID], f32, tag="hown2b")

            # ---- phase 0: deg -> dinv; dinv allgather; x_hat table ----
            with tc.tile_pool(name="ph0", bufs=2) as ph0:
                ewp_t = ph0.tile([128, p.nsup, p.L], f32, tag="ewp")
                nc.sync.dma_start(ewp_t[:, :, :], ewp_d[:, :, :])
                deg = ph0.tile([128, p.nsup], f32, tag="deg")
                nc.vector.tensor_reduce(
                    deg[:, :], ewp_t[:, :, :], mybir.AxisListType.X, Alu.add
                )
                nc.vector.tensor_scalar(deg[:, :], deg[:, :], 1.0, None, Alu.add)
                nc.scalar.sqrt(deg[:, :], deg[:, :])
                nc.vector.reciprocal(dinv_own[:, :], deg[:, :])
                nc.sync.dma_start(
                    dvin_d[:].rearrange("(s q) -> q s", q=128), dinv_own[:, :]
                )
                nc.gpsimd.collective_compute(
                    "AllGather",
                    Alu.bypass,
                    replica_groups=rg,
                    ins=[dvin_d[:]],
                    outs=[dvfull_d[:]],
                )
                dinv_full = ph0.tile([128, ncols], f32, tag="dinv_full")
                nc.sync.dma_start(
                    dinv_full[:, :], dvfull_d[:].rearrange("(s q) -> q s", q=128)
                )
                # x2own = x_own * dinv^2
                xo = ph0.tile([128, p.nsup, F_PAD], f32, tag="xo")
                nc.sync.dma_start(
                    xo[:, :, :], xown_d[:, :].rearrange("(s q) f -> q s f", q=128)
                )
                dvb = bc(dinv_own[:, :], F_PAD)
                nc.vector.tensor_tensor(x2own[:, :, :], xo[:, :, :], dvb, Alu.mult)
                nc.vector.tensor_tensor(x2own[:, :, :], x2own[:, :, :], dvb, Alu.mult)
                # x_hat table ([:, 0:F_PAD] of table1): chunks of 64 cols
                CH = 64
                for t0 in range(0, ncols, CH):
                    tch = min(CH, ncols - t0)
                    xt = ph0.tile([128, CH, F_PAD], f32, tag="xt")
                    nc.sync.dma_start(
                        xt[:, :tch, :],
                        xpad_d[:, :].rearrange("(t q) f -> q t f", q=128)[
                            :, t0 : t0 + tch, :
                        ],
                    )
                    xh = ph0.tile([128, CH, F_PAD], f16, tag="xh")
                    nc.vector.tensor_tensor(
                        xh[:, :tch, :],
                        xt[:, :tch, :],
                        bc(dinv_full[:, t0 : t0 + tch], F_PAD),
                        Alu.mult,
                    )
                    nc.sync.dma_start(
                        table1_d[:, :].rearrange("(t q) f -> q t f", q=128)[
                            :, t0 : t0 + tch, 0:F_PAD
                        ],
                        xh[:, :tch, :],
                    )

            # ---- persistent PSUM tiles ----
            pacc = psacc.tile([128, G], f32, tag="pacc")

            # ---- layers ----
            layers = [
                (0, table1_d, w1, b1, True, table2_d, None, hown2a),
                (1, table2_d, w2, b2, True, table3_d, hown2a, hown2b),
                (2, table3_d, w3, b3, False, None, hown2b, None),
            ]
            for li, tab_d, w_sb, b_sb, relu, tab_next, hin, hout in layers:
                fdim = F_PAD if li == 0 else HID
                lo_view = tab_d[0 : min(p.npad, IDX_CAP), :]
                hi_view = tab_d[p.base_hi : p.npad, :]
                for s0, nsg, chunk_ids, _tr in p.groups:
                    # one PSUM bank per supertile: windows of a supertile sit
                    # on disjoint partitions, so their has_written groups can
                    # interleave freely; supertiles never share a bank
                    aggs = [
                        psagg.tile([128, HID], f32, tag="agg", name="agg")
                        for _ in range(nsg)
                    ]
                    for ci in chunk_ids:
                        t0, ntl, cls = p.chunks[ci]
                        gt = gpool.tile([128, CHUNK_MAX, HID], f16, tag="gath")
                        view = hi_view if cls == 1 else lo_view
                        nc.gpsimd.dma_gather(
                            gt[:, :ntl, :],
                            view,
                            idx_sb[:, t0 * 8 : (t0 + ntl) * 8],
                            ntl * 128,
                            ntl * 128,
                            HID,
                            elem_step=HID,
                            single_packet=SINGLE_PACKET,
                        )
                        st = spool.tile([128, CHUNK_MAX, WIN], f16, tag="sbld")
                        nc.vector.tensor_tensor(
                            st[:, :ntl, :],
                            iota[:, :ntl, :],
                            bc(dstslot[:, t0 : t0 + ntl], WIN),
                            Alu.is_equal,
                        )
                        nc.vector.tensor_tensor(
                            st[:, :ntl, :],
                            st[:, :ntl, :],
                            bc(ews[:, t0 : t0 + ntl], WIN),
                            Alu.mult,
                        )
                        for j in range(ntl):
                            t = t0 + j
                            w, _cls = p.tiles[t]
                            sj = (w // 4) - s0
                            pb = (w % 4) * WIN
                            nc.tensor.matmul(
                                aggs[sj][pb : pb + WIN, 0:fdim],
                                st[:, j, :],
                                gt[:, j, 0:fdim],
                                start=bool(p.first_of_win[t]),
                                stop=bool(p.last_of_win[t]),
                                tile_position=(0, pb),
                                skip_group_check=True,
                            )
                    for sj in range(nsg):
                        s = s0 + sj
                        psum_agg = aggs[sj][:, 0:fdim]
                        stg = psstg.tile([128, 384], f32, tag="stg")
                        uTps = stg[:, 0:128]
                        hTps = stg[:, 128:256]
                        hbps = stg[:, 256:384]
                        dv = dinv_own[:, s : s + 1]
                        u = stpool.tile([128, HID], f32, tag="u")
                        if li == 0:
                            nc.vector.scalar_tensor_tensor(
                                u[:, 0:F_PAD],
                                psum_agg,
                                dv,
                                x2own[:, s, :],
                                Alu.mult,
                                Alu.add,
                            )
                        else:
                            nc.vector.scalar_tensor_tensor(
                                u[:, :],
                                psum_agg,
                                dv,
                                hin[:, s * HID : (s + 1) * HID],
                                Alu.mult,
                                Alu.add,
                            )
                        nc.tensor.transpose(
                            uTps[0:fdim, :], u[:, 0:fdim], ident[:, :]
                        )
                        uT = stpool.tile([128, 128], f32, tag="uTs")
                        nc.vector.tensor_copy(uT[0:fdim, :], uTps[0:fdim, :])
                        nc.tensor.matmul(
                            hTps,
                            w_sb[0:fdim, :],
                            uT[0:fdim, :],
                            start=True,
                            stop=True,
                        )
                        hT = stpool.tile([128, 128], f32, tag="hTs")
                        if relu:
                            nc.scalar.activation(
                                hT[:, :],
                                hTps,
                                Act.Relu,
                                bias=b_sb[:, 0:1],
                            )
                        else:
                            nc.vector.tensor_scalar(
                                hT[:, :], hTps, b_sb[:, 0:1], None, Alu.add
                            )
                        nc.tensor.transpose(hbps, hT[:, :], ident[:, :])
                        if li < 2:
                            hf = stpool.tile([128, 128], f16, tag="hf")
                            nc.vector.tensor_scalar(
                                hf[:, :], hbps, dv, None, Alu.mult
                            )
                            nc.vector.tensor_scalar(
                                hout[:, s * HID : (s + 1) * HID],
                                hbps,
                                dv,
                                dv,
                                Alu.mult,
                                Alu.mult,
                            )
                            nc.sync.dma_start(
                                agin_d[:, :].rearrange("(t q) f -> q t f", q=128)[
                                    :, s, :
                                ],
                                hf[:, :],
                            )
                        else:
                            h3 = stpool.tile([128, 128], f16, tag="hf")
                            nc.vector.tensor_copy(h3[:, :], hbps)
                            nc.tensor.matmul(
                                pacc[:, 0:G],
                                h3[:, :],
                                pmat[:, s * G : (s + 1) * G],
                                start=(s == 0),
                                stop=(s == p.nsup - 1),
                                skip_group_check=True,
                            )
                if tab_next is not None:
                    nc.gpsimd.collective_compute(
                        "AllGather",
                        Alu.bypass,
                        replica_groups=rg,
                        ins=[agin_d[:, :]],
                        outs=[tab_next[:, :]],
                    )

            # ---- pooling finalize + classifier ----
            pooledT = stpool.tile([128, G], f32, tag="pool")
            nc.vector.tensor_copy(pooledT[:, :], pacc[:, 0:G])
            nc.sync.dma_start(arin_d[:, :], pooledT[:, :])
            nc.gpsimd.collective_compute(
                "AllReduce",
                Alu.add,
                replica_groups=rg,
                ins=[arin_d[:, :]],
                outs=[arout_d[:, :]],
            )
            pooled2 = stpool.tile([128, G], f32, tag="pool")
            nc.sync.dma_start(pooled2[:, :], arout_d[:, :])
            lgps = psstg.tile([128, 384], f32, tag="stg")
            nc.tensor.matmul(
                lgps[0:G, 0:N_CLASS], pooled2[:, :], wl[:, :], start=True, stop=True
            )
            outt = stpool.tile([G, N_CLASS], f32, tag="out")
            nc.vector.scalar_tensor_tensor(
                outt[:, :],
                lgps[0:G, 0:N_CLASS],
                invc[:, 0:1],
                blrep[:, :],
                Alu.mult,
                Alu.add,
            )
            nc.sync.dma_start(out_d[:, :], outt[:, :])

    nc.compile()
    return nc


def make_in_maps(p, wa):
    maps = []
    for c in range(p.n_cores):
        pc = p.per_core[c]
        maps.append(
            dict(
                xpad=p.xpad,
                x_own=p.x_own[c],
                ewp=pc["ewp"].reshape(128, p.nsup, p.L),
                idx=pc["idx128"],
                dstslot=pc["dstslot"],
                ews=pc["ews"],
                iota=p.iota.reshape(128, CHUNK_MAX, WIN),
                pmat=p.pmat[c],
                w1=wa["w1"],
                w2=wa["w2"],
                w3=wa["w3"],
                wl=wa["wl"],
                b1=wa["b1"],
                b2=wa["b2"],
                b3=wa["b3"],
                invc=wa["invc"],
                blrep=wa["blrep"],
                ident=p.identity,
            )
        )
    return maps


_CACHE = {}


def kernel(x, edge_index, edge_attr, batch, W1, b1, W2, b2, W3, b3, Wl, bl):
    x = np.asarray(x)
    p = build_plan(x, np.asarray(edge_index), np.asarray(edge_attr), np.asarray(batch))
    wa = build_weight_arrays(p, W1, b1, W2, b2, W3, b3, Wl, bl)
    key = (p.n, p.tot)
    if key not in _CACHE:
        _CACHE[key] = build_program(p)
    nc = _CACHE[key]
    from concourse.bass_utils import run_bass_kernel_spmd

    res = run_bass_kernel_spmd(nc, make_in_maps(p, wa), core_ids=list(range(p.n_cores)))
    return np.asarray(res.results[0]["out"], dtype=np.float32)



# revision 4
# speedup vs baseline: 1.4646x; 1.4646x over previous
"""GCN (3x GCNConv + global mean pool + linear) on 8 Trainium2 NeuronCores.

Strategy (dst-sharded message passing, v2):
  - Nodes sharded n/8 per core; each core's nodes permuted into windows of
    32 (degree-balanced) -> supertiles of 128 (PSUM tiles).
  - Edges partitioned by dst core and packed into (window, class) tiles of
    128 edges; class = which signed-int16-indexable half of the node table
    the src row lives in (dma_gather indices are int16).
  - Normalization dinv = rsqrt(deg+1) is computed on host and folded into
    host-built scatter tiles S [128e, 32d] fp16 (coef = dinv_src*ew*dinv_dst),
    kept SBUF-resident for all three layers. Tables store PLAIN h (fp16).
  - Per layer: dma_gather pulls 256B fp16 rows of the node table from HBM,
    rotating over SWDGE queues 1..3 so three Q7 core-pairs generate DMA
    descriptors concurrently (queue 0 gathers run synchronously on the Pool
    engine; 1..3 retire early and overlap); TensorE computes the
    scatter-add as S^T @ M matmuls accumulated per window in PSUM.
  - Per supertile: u = agg + hown2 (hown2 = h*dinv^2, resident in SBUF),
    PE transpose, f32 GEMM with W, bias(+relu) on ACT, transpose back,
    fp16 table write. Tables distributed with AllGather.
  - Pooling: matmul with host-built P (1[batch==g]) accumulated over
    supertiles -> AllReduce -> final linear on-device -> out [64, 5] f32.
"""

import os
import sys
import numpy as np

for _p in ("/opt/trn_rl_repo", "/root/.axon_site/_ro/trn_rl_repo"):
    if os.path.isdir(_p) and _p not in sys.path:
        sys.path.insert(0, _p)

N_CORES = 8
N_GRAPHS = 64
HID = 128
N_CLASS = 5
F_IN = 7
F_PAD = 8
WIN = 32
SUP = 128
GROUP_SUPS = 3
CHUNK_MAX = 32
GATH_BUFS = 5
SINGLE_PACKET = False
IDX_CAP = 32768
GATHER_QUEUES = (1, 2, 3)


def _group_ranks(keys, n_keys):
    """rank of each element within its key group (keys int array)."""
    nk = len(keys)
    if nk == 0:
        return np.zeros(0, dtype=np.int64)
    order = np.argsort(keys, kind="stable")
    sk = keys[order]
    is_new = np.r_[True, sk[1:] != sk[:-1]]
    gs_idx = np.nonzero(is_new)[0]
    gs = np.repeat(gs_idx, np.diff(np.r_[gs_idx, nk]))
    rank = np.empty(nk, dtype=np.int64)
    rank[order] = np.arange(nk) - gs
    return rank


class Plan:
    pass


def build_plan(x, edge_index, edge_attr, batch, n_cores=N_CORES, n_graphs=N_GRAPHS):
    """Host-side sharding/layout planning + normalization coefficients."""
    p = Plan()
    n = x.shape[0]
    assert n % n_cores == 0
    npc = n // n_cores
    nsup = (npc + SUP - 1) // SUP
    padc = nsup * SUP
    npad = n_cores * padc
    nwin = padc // WIN
    p.n, p.npc, p.nsup, p.padc, p.npad, p.nwin = n, npc, nsup, padc, npad, nwin
    p.n_cores, p.n_graphs = n_cores, n_graphs
    base_hi = max(0, npad - IDX_CAP)
    p.base_hi = base_hi

    src = np.asarray(edge_index[0], dtype=np.int64)
    dst = np.asarray(edge_index[1], dtype=np.int64)
    ew = np.asarray(edge_attr, dtype=np.float32)
    batch = np.asarray(batch, dtype=np.int64)

    # ---- normalization (host): deg = sum of incoming ew + 1 (self loop) ----
    deg = np.bincount(dst, weights=ew.astype(np.float64), minlength=n) + 1.0
    dinv = (1.0 / np.sqrt(deg)).astype(np.float32)
    p.dinv = dinv

    # ---- window assignment (degree-balanced snake over sorted degrees) ----
    indeg = np.bincount(dst, minlength=n)
    prow = np.empty(n, dtype=np.int64)
    win_all = np.empty(n, dtype=np.int64)
    for c in range(n_cores):
        lo = c * npc
        order = np.argsort(-indeg[lo : lo + npc], kind="stable")
        pos = np.empty(npc, dtype=np.int64)
        pos[order] = np.arange(npc)
        rnd = pos // nwin
        off = pos % nwin
        w = np.where(rnd % 2 == 0, off, nwin - 1 - off)
        plid = (w // 4) * SUP + (w % 4) * WIN + rnd
        prow[lo : lo + npc] = c * padc + plid
        win_all[lo : lo + npc] = w
    p.prow = prow

    # ---- per-core-window tile budgets (uniform across cores) ----
    ecore = dst // npc
    esrc_prow = prow[src]
    can_lo = esrc_prow < IDX_CAP
    can_hi = esrc_prow >= base_hi
    ewin = win_all[dst]

    flo_cw = np.zeros((n_cores, nwin), dtype=np.int64)
    fhi_cw = np.zeros((n_cores, nwin), dtype=np.int64)
    tot_cw = np.zeros((n_cores, nwin), dtype=np.int64)
    np.add.at(tot_cw, (ecore, ewin), 1)
    np.add.at(flo_cw, (ecore[~can_hi], ewin[~can_hi]), 1)
    np.add.at(fhi_cw, (ecore[~can_lo], ewin[~can_lo]), 1)

    t_lo = np.max((flo_cw + 127) // 128, axis=0)
    t_hi = np.max((fhi_cw + 127) // 128, axis=0)
    grow = np.maximum(np.max(tot_cw, axis=0) - (t_lo + t_hi) * 128, 0)
    t_lo = t_lo + (grow + 127) // 128
    t_lo = np.maximum(t_lo, (t_lo + t_hi) == 0)
    p.t_lo, p.t_hi = t_lo, t_hi

    # ---- global tile order: groups of supertiles, class runs within group --
    n_groups = (nsup + GROUP_SUPS - 1) // GROUP_SUPS
    p.n_groups = n_groups
    tiles = []  # (win, cls)
    chunks = []  # (tile_start, ntiles, cls)
    groups = []  # (sup_start, nsups, [chunk idx], (t0, t1))
    for g in range(n_groups):
        s0 = g * GROUP_SUPS
        ns = min(GROUP_SUPS, nsup - s0)
        wlist = range(s0 * 4, (s0 + ns) * 4)
        g_t0 = len(tiles)
        g_chunks = []
        for cls in (0, 1):
            run_t0 = len(tiles)
            for w in wlist:
                tc = int(t_lo[w]) if cls == 0 else int(t_hi[w])
                tiles.extend((w, cls) for _ in range(tc))
            nrun = len(tiles) - run_t0
            t0 = run_t0
            while nrun > 0:
                take = min(CHUNK_MAX, nrun)
                g_chunks.append(len(chunks))
                chunks.append((t0, take, cls))
                t0 += take
                nrun -= take
        groups.append((s0, ns, g_chunks, (g_t0, len(tiles))))
    p.tiles, p.chunks, p.groups = tiles, chunks, groups
    p.tot = len(tiles)

    # first/last tile of each window (for PSUM start/stop flags)
    tw = np.array([t[0] for t in tiles])
    p.first_of_win = np.zeros(p.tot, dtype=bool)
    p.last_of_win = np.zeros(p.tot, dtype=bool)
    for w in range(nwin):
        ids = np.nonzero(tw == w)[0]
        p.first_of_win[ids.min()] = True
        p.last_of_win[ids.max()] = True
    # first tile index of each (win, cls) run
    t_off = {}
    for t, (w, cls) in enumerate(tiles):
        t_off.setdefault((w, cls), t)

    counts = np.bincount(batch, minlength=n_graphs).astype(np.float32)
    p.counts = counts

    # ---- per-core arrays ----
    norm = dinv[src] * ew * dinv[dst]  # full edge coefficient
    p.per_core = []
    for c in range(n_cores):
        m = ecore == c
        ed = dst[m]
        enorm = norm[m]
        eprow = esrc_prow[m]
        ewin_c = ewin[m]
        eslot = (prow[ed] % SUP) % WIN  # row within window = rnd
        e_can_hi = can_hi[m]
        e_can_lo = can_lo[m]
        ne = len(ed)

        # per-edge class: fill lo up to its target, rest hi
        ecls = np.full(ne, -1, dtype=np.int64)
        ecls[~e_can_hi] = 0
        ecls[~e_can_lo] = 1
        free = ecls == -1
        tot_w = np.bincount(ewin_c, minlength=nwin)
        flo_w = np.bincount(ewin_c[~e_can_hi], minlength=nwin)
        lo_target = np.maximum(flo_w, tot_w - t_hi * 128)
        lo_target = np.minimum(lo_target, t_lo * 128)
        fidx = np.nonzero(free)[0]
        frank = _group_ranks(ewin_c[fidx], nwin)
        to_lo = frank < (lo_target - flo_w)[ewin_c[fidx]]
        ecls[fidx[to_lo]] = 0
        ecls[fidx[~to_lo]] = 1

        # slot position within (win, cls) run
        key = ewin_c * 2 + ecls
        k = _group_ranks(key, nwin * 2)
        t_off_arr = np.zeros((nwin, 2), dtype=np.int64)
        for (wv, cv), tv in t_off.items():
            t_off_arr[wv, cv] = tv
        run0 = t_off_arr[ewin_c, ecls]
        t_of_e = run0 + k // 128
        p_of_e = k % 128

        idx_arr = np.zeros((p.tot, 128), dtype=np.int16)
        s_arr = np.zeros((p.tot, 128, WIN), dtype=np.float16)
        rel = eprow - np.where(ecls == 1, base_hi, 0)
        assert rel.min() >= 0 and rel.max() < IDX_CAP
        idx_arr[t_of_e, p_of_e] = rel.astype(np.int16)
        s_arr[t_of_e, p_of_e, eslot] = enorm.astype(np.float16)

        # wrapped idx layout [16, tot*8], replicated to [128, tot*8]
        idx16 = np.zeros((16, p.tot * 8), dtype=np.int16)
        for ppart in range(128):
            idx16[ppart % 16, np.arange(p.tot) * 8 + ppart // 16] = idx_arr[:, ppart]
        idx128 = np.ascontiguousarray(np.tile(idx16, (8, 1)))

        # S tiles SBUF layout [128 slot, tot*WIN] fp16
        s_sb = np.ascontiguousarray(
            s_arr.transpose(1, 0, 2).reshape(128, p.tot * WIN)
        )
        p.per_core.append(dict(idx128=idx128, s_sb=s_sb))

    # ---- node-indexed arrays (host layouts) ----
    xf = np.asarray(x, dtype=np.float32)
    # layer-1 table: plain x, zero-padded to [npad, HID] fp16
    tab1 = np.zeros((npad, HID), dtype=np.float16)
    tab1[prow, :F_IN] = xf.astype(np.float16)
    p.tab1 = tab1
    # per-core: x2own = x*dinv^2 [128, nsup, F_PAD] f32 and dinv2 [128, nsup]
    p.x2own = []
    p.dinv2 = []
    p.pmat = []
    for c in range(n_cores):
        lo = c * npc
        plid = prow[lo : lo + npc] - c * padc
        xo = np.zeros((128, nsup, F_PAD), dtype=np.float32)
        d2 = np.zeros((128, nsup), dtype=np.float32)
        dv2 = dinv[lo : lo + npc] * dinv[lo : lo + npc]
        xo[plid % 128, plid // 128, :F_IN] = xf[lo : lo + npc] * dv2[:, None]
        d2[plid % 128, plid // 128] = dv2
        p.x2own.append(np.ascontiguousarray(xo))
        p.dinv2.append(np.ascontiguousarray(d2))
        pm = np.zeros((128, nsup * n_graphs), dtype=np.float16)
        pm[plid % 128, (plid // 128) * n_graphs + batch[lo : lo + npc]] = 1.0
        p.pmat.append(np.ascontiguousarray(pm))

    p.identity = np.eye(128, dtype=np.float32)
    return p


def build_weight_arrays(p, W1, b1, W2, b2, W3, b3, Wl, bl):
    """Zero-pad / reshape weights (no arithmetic)."""
    w1p = np.zeros((F_PAD, HID), dtype=np.float32)
    w1p[:F_IN] = np.asarray(W1, dtype=np.float32)
    a = dict(
        w1=w1p,
        w2=np.asarray(W2, dtype=np.float32),
        w3=np.asarray(W3, dtype=np.float32),
        wl=np.asarray(Wl, dtype=np.float32),
        b1=np.asarray(b1, dtype=np.float32).reshape(HID, 1),
        b2=np.asarray(b2, dtype=np.float32).reshape(HID, 1),
        b3=np.asarray(b3, dtype=np.float32).reshape(HID, 1),
        blrep=np.ascontiguousarray(
            np.broadcast_to(np.asarray(bl, dtype=np.float32), (p.n_graphs, N_CLASS))
        ),
        invc=(1.0 / np.maximum(p.counts, 1.0)).reshape(p.n_graphs, 1),
    )
    return a


# ----------------------------------------------------------------------------
# Device program
# ----------------------------------------------------------------------------
def build_program(p, enable_asserts=False):
    import concourse.bass as bass
    import concourse.bacc as bacc
    import concourse.tile as tile
    import concourse.mybir as mybir

    dt = mybir.dt
    f32, f16, i16 = dt.float32, dt.float16, dt.int16
    Alu = mybir.AluOpType
    Act = mybir.ActivationFunctionType
    G = p.n_graphs
    rg = [list(range(p.n_cores))]

    nc = bacc.Bacc(
        "TRN2",
        target_bir_lowering=False,
        debug=False,
        enable_asserts=enable_asserts,
        num_devices=p.n_cores,
        num_swdge_queues=4,
    )

    # ---- DRAM tensors ----
    tab1_d = nc.dram_tensor("tab1", [p.npad, HID], f16, kind="ExternalInput")
    x2own_d = nc.dram_tensor("x2own", [128, p.nsup, F_PAD], f32, kind="ExternalInput")
    dinv2_d = nc.dram_tensor("dinv2", [128, p.nsup], f32, kind="ExternalInput")
    idx_d = nc.dram_tensor("idx", [128, p.tot * 8], i16, kind="ExternalInput")
    s_d = nc.dram_tensor("s_sb", [128, p.tot * WIN], f16, kind="ExternalInput")
    pmat_d = nc.dram_tensor("pmat", [128, p.nsup * G], f16, kind="ExternalInput")
    w1_d = nc.dram_tensor("w1", [F_PAD, HID], f32, kind="ExternalInput")
    w2_d = nc.dram_tensor("w2", [HID, HID], f32, kind="ExternalInput")
    w3_d = nc.dram_tensor("w3", [HID, HID], f32, kind="ExternalInput")
    wl_d = nc.dram_tensor("wl", [HID, N_CLASS], f32, kind="ExternalInput")
    b1_d = nc.dram_tensor("b1", [HID, 1], f32, kind="ExternalInput")
    b2_d = nc.dram_tensor("b2", [HID, 1], f32, kind="ExternalInput")
    b3_d = nc.dram_tensor("b3", [HID, 1], f32, kind="ExternalInput")
    invc_d = nc.dram_tensor("invc", [G, 1], f32, kind="ExternalInput")
    blrep_d = nc.dram_tensor("blrep", [G, N_CLASS], f32, kind="ExternalInput")
    ident_d = nc.dram_tensor("ident", [128, 128], f32, kind="ExternalInput")
    out_d = nc.dram_tensor("out", [G, N_CLASS], f32, kind="ExternalOutput")

    agin_d = nc.dram_tensor("agin", [p.padc, HID], f16, kind="Internal")
    table2_d = nc.dram_tensor(
        "table2", [p.npad, HID], f16, kind="Internal", addr_space="Shared"
    )
    table3_d = nc.dram_tensor(
        "table3", [p.npad, HID], f16, kind="Internal", addr_space="Shared"
    )
    arin_d = nc.dram_tensor("arin", [128, G], f32, kind="Internal")
    arout_d = nc.dram_tensor(
        "arout", [128, G], f32, kind="Internal", addr_space="Shared"
    )

    with tile.TileContext(nc) as tc:
        with (
            tc.tile_pool(name="const", bufs=1) as cpool,
            tc.tile_pool(name="gath", bufs=GATH_BUFS) as gpool,
            tc.tile_pool(name="stage", bufs=3) as stpool,
            tc.tile_pool(name="psagg", bufs=GROUP_SUPS + 1, space="PSUM") as psagg,
            tc.tile_pool(name="psstg", bufs=2, space="PSUM") as psstg,
            tc.tile_pool(name="psacc", bufs=1, space="PSUM") as psacc,
        ):
            # ---- persistent SBUF tiles ----
            ident = cpool.tile([128, 128], f32, tag="ident")
            nc.sync.dma_start(ident[:, :], ident_d[:, :])
            w1 = cpool.tile([F_PAD, HID], f32, tag="w1")
            nc.sync.dma_start(w1[:, :], w1_d[:, :])
            w2 = cpool.tile([HID, HID], f32, tag="w2")
            nc.sync.dma_start(w2[:, :], w2_d[:, :])
            w3 = cpool.tile([HID, HID], f32, tag="w3")
            nc.sync.dma_start(w3[:, :], w3_d[:, :])
            wl = cpool.tile([HID, N_CLASS], f32, tag="wl")
            nc.sync.dma_start(wl[:, :], wl_d[:, :])
            b1 = cpool.tile([HID, 1], f32, tag="b1")
            nc.sync.dma_start(b1[:, :], b1_d[:, :])
            b2 = cpool.tile([HID, 1], f32, tag="b2")
            nc.sync.dma_start(b2[:, :], b2_d[:, :])
            b3 = cpool.tile([HID, 1], f32, tag="b3")
            nc.sync.dma_start(b3[:, :], b3_d[:, :])
            invc = cpool.tile([G, 1], f32, tag="invc")
            nc.sync.dma_start(invc[:, :], invc_d[:, :])
            blrep = cpool.tile([G, N_CLASS], f32, tag="blrep")
            nc.sync.dma_start(blrep[:, :], blrep_d[:, :])
            pmat = cpool.tile([128, p.nsup * G], f16, tag="pmat")
            nc.sync.dma_start(pmat[:, :], pmat_d[:, :])
            idx_sb = cpool.tile([128, p.tot * 8], i16, tag="idx")
            nc.sync.dma_start(idx_sb[:, :], idx_d[:, :])
            s_sb = cpool.tile([128, p.tot * WIN], f16, tag="s_sb")
            nc.sync.dma_start(s_sb[:, :], s_d[:, :])
            dinv2 = cpool.tile([128, p.nsup], f32, tag="dinv2")
            nc.sync.dma_start(dinv2[:, :], dinv2_d[:, :])
            x2own = cpool.tile([128, p.nsup, F_PAD], f32, tag="x2own")
            nc.sync.dma_start(x2own[:, :, :], x2own_d[:, :, :])
            hown2a = cpool.tile([128, p.nsup * HID], f16, tag="hown2a")
            hown2b = cpool.tile([128, p.nsup * HID], f16, tag="hown2b")

            # ---- persistent PSUM tiles ----
            pacc = psacc.tile([128, G], f32, tag="pacc")

            # ---- layers ----
            layers = [
                (0, tab1_d, w1, b1, True, table2_d, None, hown2a),
                (1, table2_d, w2, b2, True, table3_d, hown2a, hown2b),
                (2, table3_d, w3, b3, False, None, hown2b, None),
            ]
            qi = 0
            for li, tab_d, w_sb, b_sb, relu, tab_next, hin, hout in layers:
                fdim = F_PAD if li == 0 else HID
                lo_view = tab_d[0 : min(p.npad, IDX_CAP), :]
                hi_view = tab_d[p.base_hi : p.npad, :]
                for s0, nsg, chunk_ids, _tr in p.groups:
                    aggs = [
                        psagg.tile([128, HID], f32, tag="agg", name="agg")
                        for _ in range(nsg)
                    ]
                    for ci in chunk_ids:
                        t0, ntl, cls = p.chunks[ci]
                        gt = gpool.tile([128, CHUNK_MAX, HID], f16, tag="gath")
                        view = hi_view if cls == 1 else lo_view
                        nc.gpsimd.dma_gather(
                            gt[:, :ntl, :],
                            view,
                            idx_sb[:, t0 * 8 : (t0 + ntl) * 8],
                            ntl * 128,
                            ntl * 128,
                            HID,
                            elem_step=HID,
                            single_packet=SINGLE_PACKET,
                            queue_num=GATHER_QUEUES[qi % len(GATHER_QUEUES)],
                        )
                        qi += 1
                        for j in range(ntl):
                            t = t0 + j
                            w, _cls = p.tiles[t]
                            sj = (w // 4) - s0
                            pb = (w % 4) * WIN
                            nc.tensor.matmul(
                                aggs[sj][pb : pb + WIN, 0:fdim],
                                s_sb[:, t * WIN : (t + 1) * WIN],
                                gt[:, j, 0:fdim],
                                start=bool(p.first_of_win[t]),
                                stop=bool(p.last_of_win[t]),
                                tile_position=(0, pb),
                                skip_group_check=True,
                            )
                    for sj in range(nsg):
                        s = s0 + sj
                        psum_agg = aggs[sj][:, 0:fdim]
                        stg = psstg.tile([128, 384], f32, tag="stg")
                        uTps = stg[:, 0:128]
                        hTps = stg[:, 128:256]
                        hbps = stg[:, 256:384]
                        u = stpool.tile([128, HID], f32, tag="u")
                        if li == 0:
                            nc.vector.tensor_tensor(
                                u[:, 0:F_PAD],
                                psum_agg,
                                x2own[:, s, :],
                                Alu.add,
                            )
                        else:
                            nc.vector.tensor_tensor(
                                u[:, :],
                                psum_agg,
                                hin[:, s * HID : (s + 1) * HID],
                                Alu.add,
                            )
                        nc.tensor.transpose(
                            uTps[0:fdim, :], u[:, 0:fdim], ident[:, :]
                        )
                        uT = stpool.tile([128, 128], f32, tag="uTs")
                        nc.vector.tensor_copy(uT[0:fdim, :], uTps[0:fdim, :])
                        nc.tensor.matmul(
                            hTps,
                            w_sb[0:fdim, :],
                            uT[0:fdim, :],
                            start=True,
                            stop=True,
                        )
                        hT = stpool.tile([128, 128], f32, tag="hTs")
                        if relu:
                            nc.scalar.activation(
                                hT[:, :],
                                hTps,
                                Act.Relu,
                                bias=b_sb[:, 0:1],
                            )
                        else:
                            nc.vector.tensor_scalar(
                                hT[:, :], hTps, b_sb[:, 0:1], None, Alu.add
                            )
                        nc.tensor.transpose(hbps, hT[:, :], ident[:, :])
                        if li < 2:
                            hf = stpool.tile([128, 128], f16, tag="hf")
                            nc.vector.tensor_copy(hf[:, :], hbps)
                            nc.vector.tensor_scalar(
                                hout[:, s * HID : (s + 1) * HID],
                                hbps,
                                dinv2[:, s : s + 1],
                                None,
                                Alu.mult,
                            )
                            nc.sync.dma_start(
                                agin_d[:, :].rearrange("(t q) f -> q t f", q=128)[
                                    :, s, :
                                ],
                                hf[:, :],
                            )
                        else:
                            h3 = stpool.tile([128, 128], f16, tag="hf")
                            nc.vector.tensor_copy(h3[:, :], hbps)
                            nc.tensor.matmul(
                                pacc[:, 0:G],
                                h3[:, :],
                                pmat[:, s * G : (s + 1) * G],
                                start=(s == 0),
                                stop=(s == p.nsup - 1),
                                skip_group_check=True,
                            )
                if tab_next is not None:
                    nc.gpsimd.collective_compute(
                        "AllGather",
                        Alu.bypass,
                        replica_groups=rg,
                        ins=[agin_d[:, :]],
                        outs=[tab_next[:, :]],
                    )

            # ---- pooling finalize + classifier ----
            pooledT = stpool.tile([128, G], f32, tag="pool")
            nc.vector.tensor_copy(pooledT[:, :], pacc[:, 0:G])
            nc.sync.dma_start(arin_d[:, :], pooledT[:, :])
            nc.gpsimd.collective_compute(
                "AllReduce",
                Alu.add,
                replica_groups=rg,
                ins=[arin_d[:, :]],
                outs=[arout_d[:, :]],
            )
            pooled2 = stpool.tile([128, G], f32, tag="pool")
            nc.sync.dma_start(pooled2[:, :], arout_d[:, :])
            lgps = psstg.tile([128, 384], f32, tag="stg")
            nc.tensor.matmul(
                lgps[0:G, 0:N_CLASS], pooled2[:, :], wl[:, :], start=True, stop=True
            )
            outt = stpool.tile([G, N_CLASS], f32, tag="out")
            nc.vector.scalar_tensor_tensor(
                outt[:, :],
                lgps[0:G, 0:N_CLASS],
                invc[:, 0:1],
                blrep[:, :],
                Alu.mult,
                Alu.add,
            )
            nc.sync.dma_start(out_d[:, :], outt[:, :])

    nc.compile()
    return nc


def make_in_maps(p, wa):
    maps = []
    for c in range(p.n_cores):
        pc = p.per_core[c]
        maps.append(
            dict(
                tab1=p.tab1,
                x2own=p.x2own[c],
                dinv2=p.dinv2[c],
                idx=pc["idx128"],
                s_sb=pc["s_sb"],
                pmat=p.pmat[c],
                w1=wa["w1"],
                w2=wa["w2"],
                w3=wa["w3"],
                wl=wa["wl"],
                b1=wa["b1"],
                b2=wa["b2"],
                b3=wa["b3"],
                invc=wa["invc"],
                blrep=wa["blrep"],
                ident=p.identity,
            )
        )
    return maps


_CACHE = {}


def kernel(x, edge_index, edge_attr, batch, W1, b1, W2, b2, W3, b3, Wl, bl):
    x = np.asarray(x)
    p = build_plan(x, np.asarray(edge_index), np.asarray(edge_attr), np.asarray(batch))
    wa = build_weight_arrays(p, W1, b1, W2, b2, W3, b3, Wl, bl)
    key = (p.n, p.tot)
    if key not in _CACHE:
        _CACHE[key] = build_program(p)
    nc = _CACHE[key]
    from concourse.bass_utils import run_bass_kernel_spmd

    res = run_bass_kernel_spmd(nc, make_in_maps(p, wa), core_ids=list(range(p.n_cores)))
    return np.asarray(res.results[0]["out"], dtype=np.float32)


# revision 10
# speedup vs baseline: 1.5168x; 1.0357x over previous
"""GCN (3x GCNConv + global mean pool + linear) on 8 Trainium2 NeuronCores.

Strategy (dst-sharded message passing, v2):
  - Nodes sharded n/8 per core; each core's nodes permuted into windows of
    32 (degree-balanced) -> supertiles of 128 (PSUM tiles).
  - Edges partitioned by dst core and packed into (window, class) tiles of
    128 edges; class = which signed-int16-indexable half of the node table
    the src row lives in (dma_gather indices are int16).
  - Normalization dinv = rsqrt(deg+1) is computed on host and folded into
    host-built scatter tiles S [128e, 32d] fp16 (coef = dinv_src*ew*dinv_dst),
    kept SBUF-resident for all three layers. Tables store PLAIN h (fp16).
  - Per layer: dma_gather pulls 256B fp16 rows of the node table from HBM,
    rotating over SWDGE queues 1..3 so three Q7 core-pairs generate DMA
    descriptors concurrently (queue 0 gathers run synchronously on the Pool
    engine; 1..3 retire early and overlap); TensorE computes the
    scatter-add as S^T @ M matmuls accumulated per window in PSUM.
  - Per supertile: u = agg + hown2 (hown2 = h*dinv^2, resident in SBUF),
    PE transpose, f32 GEMM with W, bias(+relu) on ACT, transpose back,
    fp16 table write. Tables distributed with AllGather.
  - Pooling: matmul with host-built P (1[batch==g]) accumulated over
    supertiles -> AllReduce -> final linear on-device -> out [64, 5] f32.
"""

import os
import sys
import numpy as np

for _p in ("/opt/trn_rl_repo", "/root/.axon_site/_ro/trn_rl_repo"):
    if os.path.isdir(_p) and _p not in sys.path:
        sys.path.insert(0, _p)

N_CORES = 8
N_GRAPHS = 64
HID = 128
N_CLASS = 5
F_IN = 7
F_PAD = 8
WIN = 32
SUP = 128
GROUP_SUPS = 3
CHUNK_MAX = 32
GATH_BUFS = 6
SINGLE_PACKET = False
IDX_CAP = 32768
GATHER_QUEUES = (1, 2, 3)


def _group_ranks(keys, n_keys):
    """rank of each element within its key group (keys int array)."""
    nk = len(keys)
    if nk == 0:
        return np.zeros(0, dtype=np.int64)
    order = np.argsort(keys, kind="stable")
    sk = keys[order]
    is_new = np.r_[True, sk[1:] != sk[:-1]]
    gs_idx = np.nonzero(is_new)[0]
    gs = np.repeat(gs_idx, np.diff(np.r_[gs_idx, nk]))
    rank = np.empty(nk, dtype=np.int64)
    rank[order] = np.arange(nk) - gs
    return rank


class Plan:
    pass


def build_plan(x, edge_index, edge_attr, batch, n_cores=N_CORES, n_graphs=N_GRAPHS):
    """Host-side sharding/layout planning + normalization coefficients."""
    p = Plan()
    n = x.shape[0]
    assert n % n_cores == 0
    npc = n // n_cores
    nsup = (npc + SUP - 1) // SUP
    padc = nsup * SUP
    npad = n_cores * padc
    nwin = padc // WIN
    p.n, p.npc, p.nsup, p.padc, p.npad, p.nwin = n, npc, nsup, padc, npad, nwin
    p.n_cores, p.n_graphs = n_cores, n_graphs
    base_hi = max(0, npad - IDX_CAP)
    p.base_hi = base_hi

    src = np.asarray(edge_index[0], dtype=np.int64)
    dst = np.asarray(edge_index[1], dtype=np.int64)
    ew = np.asarray(edge_attr, dtype=np.float32)
    batch = np.asarray(batch, dtype=np.int64)

    # ---- normalization (host): deg = sum of incoming ew + 1 (self loop) ----
    deg = np.bincount(dst, weights=ew.astype(np.float64), minlength=n) + 1.0
    dinv = (1.0 / np.sqrt(deg)).astype(np.float32)
    p.dinv = dinv

    # ---- window assignment (degree-balanced snake over sorted degrees) ----
    indeg = np.bincount(dst, minlength=n)
    prow = np.empty(n, dtype=np.int64)
    win_all = np.empty(n, dtype=np.int64)
    for c in range(n_cores):
        lo = c * npc
        order = np.argsort(-indeg[lo : lo + npc], kind="stable")
        pos = np.empty(npc, dtype=np.int64)
        pos[order] = np.arange(npc)
        rnd = pos // nwin
        off = pos % nwin
        w = np.where(rnd % 2 == 0, off, nwin - 1 - off)
        plid = (w // 4) * SUP + (w % 4) * WIN + rnd
        prow[lo : lo + npc] = c * padc + plid
        win_all[lo : lo + npc] = w
    p.prow = prow

    # ---- per-core-window tile budgets (uniform across cores) ----
    ecore = dst // npc
    esrc_prow = prow[src]
    can_lo = esrc_prow < IDX_CAP
    can_hi = esrc_prow >= base_hi
    ewin = win_all[dst]

    flo_cw = np.zeros((n_cores, nwin), dtype=np.int64)
    fhi_cw = np.zeros((n_cores, nwin), dtype=np.int64)
    tot_cw = np.zeros((n_cores, nwin), dtype=np.int64)
    np.add.at(tot_cw, (ecore, ewin), 1)
    np.add.at(flo_cw, (ecore[~can_hi], ewin[~can_hi]), 1)
    np.add.at(fhi_cw, (ecore[~can_lo], ewin[~can_lo]), 1)

    t_lo = np.max((flo_cw + 127) // 128, axis=0)
    t_hi = np.max((fhi_cw + 127) // 128, axis=0)
    grow = np.maximum(np.max(tot_cw, axis=0) - (t_lo + t_hi) * 128, 0)
    t_lo = t_lo + (grow + 127) // 128
    t_lo = np.maximum(t_lo, (t_lo + t_hi) == 0)
    p.t_lo, p.t_hi = t_lo, t_hi

    # ---- global tile order: groups of supertiles, class runs within group --
    n_groups = (nsup + GROUP_SUPS - 1) // GROUP_SUPS
    p.n_groups = n_groups
    tiles = []  # (win, cls)
    chunks = []  # (tile_start, ntiles, cls)
    groups = []  # (sup_start, nsups, [chunk idx], (t0, t1))
    for g in range(n_groups):
        s0 = g * GROUP_SUPS
        ns = min(GROUP_SUPS, nsup - s0)
        wlist = range(s0 * 4, (s0 + ns) * 4)
        g_t0 = len(tiles)
        g_chunks = []
        for cls in (0, 1):
            run_t0 = len(tiles)
            for w in wlist:
                tc = int(t_lo[w]) if cls == 0 else int(t_hi[w])
                tiles.extend((w, cls) for _ in range(tc))
            nrun = len(tiles) - run_t0
            t0 = run_t0
            while nrun > 0:
                take = min(CHUNK_MAX, nrun)
                g_chunks.append(len(chunks))
                chunks.append((t0, take, cls))
                t0 += take
                nrun -= take
        groups.append((s0, ns, g_chunks, (g_t0, len(tiles))))
    p.tiles, p.chunks, p.groups = tiles, chunks, groups
    p.tot = len(tiles)

    # queue assignment: greedy least-loaded (by tile count) over queues 1..3
    qload = {q: 0 for q in GATHER_QUEUES}
    p.chunk_queue = []
    for _t0, ntl, _cls in chunks:
        q = min(qload, key=lambda k: qload[k])
        p.chunk_queue.append(q)
        qload[q] += ntl

    # first/last tile of each window (for PSUM start/stop flags)
    tw = np.array([t[0] for t in tiles])
    p.first_of_win = np.zeros(p.tot, dtype=bool)
    p.last_of_win = np.zeros(p.tot, dtype=bool)
    for w in range(nwin):
        ids = np.nonzero(tw == w)[0]
        p.first_of_win[ids.min()] = True
        p.last_of_win[ids.max()] = True
    # first tile index of each (win, cls) run
    t_off = {}
    for t, (w, cls) in enumerate(tiles):
        t_off.setdefault((w, cls), t)

    counts = np.bincount(batch, minlength=n_graphs).astype(np.float32)
    p.counts = counts

    # ---- per-core arrays ----
    norm = dinv[src] * ew * dinv[dst]  # full edge coefficient
    p.per_core = []
    for c in range(n_cores):
        m = ecore == c
        ed = dst[m]
        enorm = norm[m]
        eprow = esrc_prow[m]
        ewin_c = ewin[m]
        eslot = (prow[ed] % SUP) % WIN  # row within window = rnd
        e_can_hi = can_hi[m]
        e_can_lo = can_lo[m]
        ne = len(ed)

        # per-edge class: fill lo up to its target, rest hi
        ecls = np.full(ne, -1, dtype=np.int64)
        ecls[~e_can_hi] = 0
        ecls[~e_can_lo] = 1
        free = ecls == -1
        tot_w = np.bincount(ewin_c, minlength=nwin)
        flo_w = np.bincount(ewin_c[~e_can_hi], minlength=nwin)
        lo_target = np.maximum(flo_w, tot_w - t_hi * 128)
        lo_target = np.minimum(lo_target, t_lo * 128)
        fidx = np.nonzero(free)[0]
        frank = _group_ranks(ewin_c[fidx], nwin)
        to_lo = frank < (lo_target - flo_w)[ewin_c[fidx]]
        ecls[fidx[to_lo]] = 0
        ecls[fidx[~to_lo]] = 1

        # slot position within (win, cls) run
        key = ewin_c * 2 + ecls
        k = _group_ranks(key, nwin * 2)
        t_off_arr = np.zeros((nwin, 2), dtype=np.int64)
        for (wv, cv), tv in t_off.items():
            t_off_arr[wv, cv] = tv
        run0 = t_off_arr[ewin_c, ecls]
        t_of_e = run0 + k // 128
        p_of_e = k % 128

        idx_arr = np.zeros((p.tot, 128), dtype=np.int16)
        s_arr = np.zeros((p.tot, 128, WIN), dtype=np.float16)
        rel = eprow - np.where(ecls == 1, base_hi, 0)
        assert rel.min() >= 0 and rel.max() < IDX_CAP
        idx_arr[t_of_e, p_of_e] = rel.astype(np.int16)
        s_arr[t_of_e, p_of_e, eslot] = enorm.astype(np.float16)

        # wrapped idx layout [16, tot*8], replicated to [128, tot*8]
        idx16 = np.zeros((16, p.tot * 8), dtype=np.int16)
        for ppart in range(128):
            idx16[ppart % 16, np.arange(p.tot) * 8 + ppart // 16] = idx_arr[:, ppart]
        idx128 = np.ascontiguousarray(np.tile(idx16, (8, 1)))

        # S tiles SBUF layout [128 slot, tot*WIN] fp16
        s_sb = np.ascontiguousarray(
            s_arr.transpose(1, 0, 2).reshape(128, p.tot * WIN)
        )
        p.per_core.append(dict(idx128=idx128, s_sb=s_sb))

    # ---- node-indexed arrays (host layouts) ----
    xf = np.asarray(x, dtype=np.float32)
    # layer-1 table: plain x, zero-padded to [npad, HID] fp16
    tab1 = np.zeros((npad, HID), dtype=np.float16)
    tab1[prow, :F_IN] = xf.astype(np.float16)
    p.tab1 = tab1
    # per-core: x2own = x*dinv^2 [128, nsup, F_PAD] f32 and dinv2 [128, nsup]
    p.x2own = []
    p.dinv2 = []
    p.pmat = []
    for c in range(n_cores):
        lo = c * npc
        plid = prow[lo : lo + npc] - c * padc
        xo = np.zeros((128, nsup, F_PAD), dtype=np.float32)
        d2 = np.zeros((128, nsup), dtype=np.float32)
        dv2 = dinv[lo : lo + npc] * dinv[lo : lo + npc]
        xo[plid % 128, plid // 128, :F_IN] = xf[lo : lo + npc] * dv2[:, None]
        d2[plid % 128, plid // 128] = dv2
        p.x2own.append(np.ascontiguousarray(xo))
        p.dinv2.append(np.ascontiguousarray(d2))
        pm = np.zeros((128, nsup * n_graphs), dtype=np.float16)
        pm[plid % 128, (plid // 128) * n_graphs + batch[lo : lo + npc]] = 1.0
        p.pmat.append(np.ascontiguousarray(pm))

    p.identity = np.eye(128, dtype=np.float32)
    return p


def build_weight_arrays(p, W1, b1, W2, b2, W3, b3, Wl, bl):
    """Zero-pad / reshape weights (no arithmetic)."""
    w1p = np.zeros((F_PAD, HID), dtype=np.float32)
    w1p[:F_IN] = np.asarray(W1, dtype=np.float32)
    a = dict(
        w1=w1p,
        w2=np.asarray(W2, dtype=np.float32),
        w3=np.asarray(W3, dtype=np.float32),
        wl=np.asarray(Wl, dtype=np.float32),
        b1=np.asarray(b1, dtype=np.float32).reshape(HID, 1),
        b2=np.asarray(b2, dtype=np.float32).reshape(HID, 1),
        b3=np.asarray(b3, dtype=np.float32).reshape(HID, 1),
        blrep=np.ascontiguousarray(
            np.broadcast_to(np.asarray(bl, dtype=np.float32), (p.n_graphs, N_CLASS))
        ),
        invc=(1.0 / np.maximum(p.counts, 1.0)).reshape(p.n_graphs, 1),
    )
    return a


# ----------------------------------------------------------------------------
# Device program
# ----------------------------------------------------------------------------
def build_program(p, enable_asserts=False):
    import concourse.bass as bass
    import concourse.bacc as bacc
    import concourse.tile as tile
    import concourse.mybir as mybir

    dt = mybir.dt
    f32, f16, i16 = dt.float32, dt.float16, dt.int16
    Alu = mybir.AluOpType
    Act = mybir.ActivationFunctionType
    G = p.n_graphs
    rg = [list(range(p.n_cores))]

    nc = bacc.Bacc(
        "TRN2",
        target_bir_lowering=False,
        debug=False,
        enable_asserts=enable_asserts,
        num_devices=p.n_cores,
        num_swdge_queues=4,
    )

    # ---- DRAM tensors ----
    tab1_d = nc.dram_tensor("tab1", [p.npad, HID], f16, kind="ExternalInput")
    x2own_d = nc.dram_tensor("x2own", [128, p.nsup, F_PAD], f32, kind="ExternalInput")
    dinv2_d = nc.dram_tensor("dinv2", [128, p.nsup], f32, kind="ExternalInput")
    idx_d = nc.dram_tensor("idx", [128, p.tot * 8], i16, kind="ExternalInput")
    s_d = nc.dram_tensor("s_sb", [128, p.tot * WIN], f16, kind="ExternalInput")
    pmat_d = nc.dram_tensor("pmat", [128, p.nsup * G], f16, kind="ExternalInput")
    w1_d = nc.dram_tensor("w1", [F_PAD, HID], f32, kind="ExternalInput")
    w2_d = nc.dram_tensor("w2", [HID, HID], f32, kind="ExternalInput")
    w3_d = nc.dram_tensor("w3", [HID, HID], f32, kind="ExternalInput")
    wl_d = nc.dram_tensor("wl", [HID, N_CLASS], f32, kind="ExternalInput")
    b1_d = nc.dram_tensor("b1", [HID, 1], f32, kind="ExternalInput")
    b2_d = nc.dram_tensor("b2", [HID, 1], f32, kind="ExternalInput")
    b3_d = nc.dram_tensor("b3", [HID, 1], f32, kind="ExternalInput")
    invc_d = nc.dram_tensor("invc", [G, 1], f32, kind="ExternalInput")
    blrep_d = nc.dram_tensor("blrep", [G, N_CLASS], f32, kind="ExternalInput")
    ident_d = nc.dram_tensor("ident", [128, 128], f32, kind="ExternalInput")
    out_d = nc.dram_tensor("out", [G, N_CLASS], f32, kind="ExternalOutput")

    agin_d = nc.dram_tensor("agin", [p.padc, HID], f16, kind="Internal")
    table2_d = nc.dram_tensor(
        "table2", [p.npad, HID], f16, kind="Internal", addr_space="Shared"
    )
    table3_d = nc.dram_tensor(
        "table3", [p.npad, HID], f16, kind="Internal", addr_space="Shared"
    )
    arin_d = nc.dram_tensor("arin", [128, G], f32, kind="Internal")
    arout_d = nc.dram_tensor(
        "arout", [128, G], f32, kind="Internal", addr_space="Shared"
    )
    warm_in_d = nc.dram_tensor("warm_in", [128, 8], f32, kind="Internal")
    warm_out_d = nc.dram_tensor(
        "warm_out", [128, 64], f32, kind="Internal", addr_space="Shared"
    )

    with tile.TileContext(nc) as tc:
        with (
            tc.tile_pool(name="const", bufs=1) as cpool,
            tc.tile_pool(name="gath", bufs=GATH_BUFS) as gpool,
            tc.tile_pool(name="stage", bufs=3) as stpool,
            tc.tile_pool(name="psagg", bufs=GROUP_SUPS + 1, space="PSUM") as psagg,
            tc.tile_pool(name="psstg", bufs=2, space="PSUM") as psstg,
            tc.tile_pool(name="psacc", bufs=1, space="PSUM") as psacc,
        ):
            # ---- persistent SBUF tiles ----
            ident = cpool.tile([128, 128], f32, tag="ident")
            nc.sync.dma_start(ident[:, :], ident_d[:, :])
            w1 = cpool.tile([F_PAD, HID], f32, tag="w1")
            nc.sync.dma_start(w1[:, :], w1_d[:, :])
            w2 = cpool.tile([HID, HID], f32, tag="w2")
            nc.sync.dma_start(w2[:, :], w2_d[:, :])
            w3 = cpool.tile([HID, HID], f32, tag="w3")
            nc.sync.dma_start(w3[:, :], w3_d[:, :])
            wl = cpool.tile([HID, N_CLASS], f32, tag="wl")
            nc.sync.dma_start(wl[:, :], wl_d[:, :])
            b1 = cpool.tile([HID, 1], f32, tag="b1")
            nc.sync.dma_start(b1[:, :], b1_d[:, :])
            b2 = cpool.tile([HID, 1], f32, tag="b2")
            nc.sync.dma_start(b2[:, :], b2_d[:, :])
            b3 = cpool.tile([HID, 1], f32, tag="b3")
            nc.sync.dma_start(b3[:, :], b3_d[:, :])
            invc = cpool.tile([G, 1], f32, tag="invc")
            nc.sync.dma_start(invc[:, :], invc_d[:, :])
            blrep = cpool.tile([G, N_CLASS], f32, tag="blrep")
            nc.sync.dma_start(blrep[:, :], blrep_d[:, :])
            pmat = cpool.tile([128, p.nsup * G], f16, tag="pmat")
            nc.sync.dma_start(pmat[:, :], pmat_d[:, :])
            idx_sb = cpool.tile([128, p.tot * 8], i16, tag="idx")
            nc.sync.dma_start(idx_sb[:, :], idx_d[:, :])
            s_sb = cpool.tile([128, p.tot * WIN], f16, tag="s_sb")
            nc.sync.dma_start(s_sb[:, :], s_d[:, :])
            dinv2 = cpool.tile([128, p.nsup], f32, tag="dinv2")
            nc.sync.dma_start(dinv2[:, :], dinv2_d[:, :])
            x2own = cpool.tile([128, p.nsup, F_PAD], f32, tag="x2own")
            nc.sync.dma_start(x2own[:, :, :], x2own_d[:, :, :])
            hown2a = cpool.tile([128, p.nsup * HID], f16, tag="hown2a")
            hown2b = cpool.tile([128, p.nsup * HID], f16, tag="hown2b")

            # warm-up collective: absorbs first-collective latency + aligns
            # the cores before the timed layers
            nc.sync.dma_start(warm_in_d[:, :], ident[:, 0:8])
            nc.gpsimd.collective_compute(
                "AllGather",
                Alu.bypass,
                replica_groups=rg,
                ins=[warm_in_d[:, :]],
                outs=[warm_out_d[:, :]],
            )

            # ---- persistent PSUM tiles ----
            pacc = psacc.tile([128, G], f32, tag="pacc")

            # ---- layers ----
            layers = [
                (0, tab1_d, w1, b1, True, table2_d, None, hown2a),
                (1, table2_d, w2, b2, True, table3_d, hown2a, hown2b),
                (2, table3_d, w3, b3, False, None, hown2b, None),
            ]
            for li, tab_d, w_sb, b_sb, relu, tab_next, hin, hout in layers:
                fdim = F_PAD if li == 0 else HID
                lo_view = tab_d[0 : min(p.npad, IDX_CAP), :]
                hi_view = tab_d[p.base_hi : p.npad, :]
                for s0, nsg, chunk_ids, _tr in p.groups:
                    aggs = [
                        psagg.tile([128, HID], f32, tag="agg", name="agg")
                        for _ in range(nsg)
                    ]
                    for ci in chunk_ids:
                        t0, ntl, cls = p.chunks[ci]
                        gt = gpool.tile([128, CHUNK_MAX, HID], f16, tag="gath")
                        view = hi_view if cls == 1 else lo_view
                        nc.gpsimd.dma_gather(
                            gt[:, :ntl, :],
                            view,
                            idx_sb[:, t0 * 8 : (t0 + ntl) * 8],
                            ntl * 128,
                            ntl * 128,
                            HID,
                            elem_step=HID,
                            single_packet=SINGLE_PACKET,
                            queue_num=p.chunk_queue[ci],
                        )
                        for j in range(ntl):
                            t = t0 + j
                            w, _cls = p.tiles[t]
                            sj = (w // 4) - s0
                            pb = (w % 4) * WIN
                            nc.tensor.matmul(
                                aggs[sj][pb : pb + WIN, 0:fdim],
                                s_sb[:, t * WIN : (t + 1) * WIN],
                                gt[:, j, 0:fdim],
                                start=bool(p.first_of_win[t]),
                                stop=bool(p.last_of_win[t]),
                                tile_position=(0, pb),
                                skip_group_check=True,
                            )
                    for sj in range(nsg):
                        s = s0 + sj
                        psum_agg = aggs[sj][:, 0:fdim]
                        stg = psstg.tile([128, 384], f32, tag="stg")
                        uTps = stg[:, 0:128]
                        hTps = stg[:, 128:256]
                        hbps = stg[:, 256:384]
                        u = stpool.tile([128, HID], f32, tag="u")
                        if li == 0:
                            nc.vector.tensor_tensor(
                                u[:, 0:F_PAD],
                                psum_agg,
                                x2own[:, s, :],
                                Alu.add,
                            )
                        else:
                            nc.vector.tensor_tensor(
                                u[:, :],
                                psum_agg,
                                hin[:, s * HID : (s + 1) * HID],
                                Alu.add,
                            )
                        nc.tensor.transpose(
                            uTps[0:fdim, :], u[:, 0:fdim], ident[:, :]
                        )
                        uT = stpool.tile([128, 128], f32, tag="uTs")
                        nc.vector.tensor_copy(uT[0:fdim, :], uTps[0:fdim, :])
                        nc.tensor.matmul(
                            hTps,
                            w_sb[0:fdim, :],
                            uT[0:fdim, :],
                            start=True,
                            stop=True,
                        )
                        hT = stpool.tile([128, 128], f32, tag="hTs")
                        if relu:
                            nc.scalar.activation(
                                hT[:, :],
                                hTps,
                                Act.Relu,
                                bias=b_sb[:, 0:1],
                            )
                        else:
                            nc.vector.tensor_scalar(
                                hT[:, :], hTps, b_sb[:, 0:1], None, Alu.add
                            )
                        nc.tensor.transpose(hbps, hT[:, :], ident[:, :])
                        if li < 2:
                            hf = stpool.tile([128, 128], f16, tag="hf")
                            nc.vector.tensor_copy(hf[:, :], hbps)
                            nc.vector.tensor_scalar(
                                hout[:, s * HID : (s + 1) * HID],
                                hbps,
                                dinv2[:, s : s + 1],
                                None,
                                Alu.mult,
                            )
                            nc.sync.dma_start(
                                agin_d[:, :].rearrange("(t q) f -> q t f", q=128)[
                                    :, s, :
                                ],
                                hf[:, :],
                            )
                        else:
                            h3 = stpool.tile([128, 128], f16, tag="hf")
                            nc.vector.tensor_copy(h3[:, :], hbps)
                            nc.tensor.matmul(
                                pacc[:, 0:G],
                                h3[:, :],
                                pmat[:, s * G : (s + 1) * G],
                                start=(s == 0),
                                stop=(s == p.nsup - 1),
                                skip_group_check=True,
                            )
                if tab_next is not None:
                    nc.gpsimd.collective_compute(
                        "AllGather",
                        Alu.bypass,
                        replica_groups=rg,
                        ins=[agin_d[:, :]],
                        outs=[tab_next[:, :]],
                    )

            # ---- pooling finalize + classifier ----
            pooledT = stpool.tile([128, G], f32, tag="pool")
            nc.vector.tensor_copy(pooledT[:, :], pacc[:, 0:G])
            nc.sync.dma_start(arin_d[:, :], pooledT[:, :])
            nc.gpsimd.collective_compute(
                "AllReduce",
                Alu.add,
                replica_groups=rg,
                ins=[arin_d[:, :]],
                outs=[arout_d[:, :]],
            )
            pooled2 = stpool.tile([128, G], f32, tag="pool")
            nc.sync.dma_start(pooled2[:, :], arout_d[:, :])
            lgps = psstg.tile([128, 384], f32, tag="stg")
            nc.tensor.matmul(
                lgps[0:G, 0:N_CLASS], pooled2[:, :], wl[:, :], start=True, stop=True
            )
            outt = stpool.tile([G, N_CLASS], f32, tag="out")
            nc.vector.scalar_tensor_tensor(
                outt[:, :],
                lgps[0:G, 0:N_CLASS],
                invc[:, 0:1],
                blrep[:, :],
                Alu.mult,
                Alu.add,
            )
            nc.sync.dma_start(out_d[:, :], outt[:, :])

    nc.compile()
    return nc


def make_in_maps(p, wa):
    maps = []
    for c in range(p.n_cores):
        pc = p.per_core[c]
        maps.append(
            dict(
                tab1=p.tab1,
                x2own=p.x2own[c],
                dinv2=p.dinv2[c],
                idx=pc["idx128"],
                s_sb=pc["s_sb"],
                pmat=p.pmat[c],
                w1=wa["w1"],
                w2=wa["w2"],
                w3=wa["w3"],
                wl=wa["wl"],
                b1=wa["b1"],
                b2=wa["b2"],
                b3=wa["b3"],
                invc=wa["invc"],
                blrep=wa["blrep"],
                ident=p.identity,
            )
        )
    return maps


_CACHE = {}


def kernel(x, edge_index, edge_attr, batch, W1, b1, W2, b2, W3, b3, Wl, bl):
    x = np.asarray(x)
    p = build_plan(x, np.asarray(edge_index), np.asarray(edge_attr), np.asarray(batch))
    wa = build_weight_arrays(p, W1, b1, W2, b2, W3, b3, Wl, bl)
    key = (p.n, p.tot)
    if key not in _CACHE:
        _CACHE[key] = build_program(p)
    nc = _CACHE[key]
    from concourse.bass_utils import run_bass_kernel_spmd

    res = run_bass_kernel_spmd(nc, make_in_maps(p, wa), core_ids=list(range(p.n_cores)))
    return np.asarray(res.results[0]["out"], dtype=np.float32)


# revision 13
# speedup vs baseline: 1.6712x; 1.1018x over previous
"""GCN (3x GCNConv + global mean pool + linear) on 8 Trainium2 NeuronCores.

Strategy (dst-sharded message passing, v2):
  - Nodes sharded n/8 per core; each core's nodes permuted into windows of
    32 (degree-balanced) -> supertiles of 128 (PSUM tiles).
  - Edges partitioned by dst core and packed into (window, class) tiles of
    128 edges; class = which signed-int16-indexable half of the node table
    the src row lives in (dma_gather indices are int16).
  - Normalization dinv = rsqrt(deg+1) is computed on host and folded into
    host-built scatter tiles S [128e, 32d] fp16 (coef = dinv_src*ew*dinv_dst),
    kept SBUF-resident for all three layers. Tables store PLAIN h (fp16).
  - Per layer: dma_gather pulls 256B fp16 rows of the node table from HBM,
    rotating over SWDGE queues 1..3 so three Q7 core-pairs generate DMA
    descriptors concurrently (queue 0 gathers run synchronously on the Pool
    engine; 1..3 retire early and overlap); TensorE computes the
    scatter-add as S^T @ M matmuls accumulated per window in PSUM.
  - Per supertile: u = agg + hown2 (hown2 = h*dinv^2, resident in SBUF),
    PE transpose, f32 GEMM with W, bias(+relu) on ACT, transpose back,
    fp16 table write. Tables distributed with AllGather.
  - Pooling: matmul with host-built P (1[batch==g]) accumulated over
    supertiles -> AllReduce -> final linear on-device -> out [64, 5] f32.
"""

import os
import sys
import numpy as np

for _p in ("/opt/trn_rl_repo", "/root/.axon_site/_ro/trn_rl_repo"):
    if os.path.isdir(_p) and _p not in sys.path:
        sys.path.insert(0, _p)

N_CORES = 8
N_GRAPHS = 64
HID = 128
N_CLASS = 5
F_IN = 7
F_PAD = 8
WIN = 32
SUP = 128
GROUP_SUPS = 3
CHUNK_MAX = 32
GATH_BUFS = 6
SINGLE_PACKET = False
IDX_CAP = 32768
GATHER_QUEUES = (1, 2, 3)


def _group_ranks(keys, n_keys):
    """rank of each element within its key group (keys int array)."""
    nk = len(keys)
    if nk == 0:
        return np.zeros(0, dtype=np.int64)
    order = np.argsort(keys, kind="stable")
    sk = keys[order]
    is_new = np.r_[True, sk[1:] != sk[:-1]]
    gs_idx = np.nonzero(is_new)[0]
    gs = np.repeat(gs_idx, np.diff(np.r_[gs_idx, nk]))
    rank = np.empty(nk, dtype=np.int64)
    rank[order] = np.arange(nk) - gs
    return rank


class Plan:
    pass


def build_plan(x, edge_index, edge_attr, batch, n_cores=N_CORES, n_graphs=N_GRAPHS):
    """Host-side sharding/layout planning + normalization coefficients."""
    p = Plan()
    n = x.shape[0]
    assert n % n_cores == 0
    npc = n // n_cores
    nsup = (npc + SUP - 1) // SUP
    padc = nsup * SUP
    npad = n_cores * padc
    nwin = padc // WIN
    p.n, p.npc, p.nsup, p.padc, p.npad, p.nwin = n, npc, nsup, padc, npad, nwin
    p.n_cores, p.n_graphs = n_cores, n_graphs
    base_hi = max(0, npad - IDX_CAP)
    p.base_hi = base_hi

    src = np.asarray(edge_index[0], dtype=np.int64)
    dst = np.asarray(edge_index[1], dtype=np.int64)
    ew = np.asarray(edge_attr, dtype=np.float32)
    batch = np.asarray(batch, dtype=np.int64)

    # ---- normalization (host): deg = sum of incoming ew + 1 (self loop) ----
    deg = np.bincount(dst, weights=ew.astype(np.float64), minlength=n) + 1.0
    dinv = (1.0 / np.sqrt(deg)).astype(np.float32)
    p.dinv = dinv

    # ---- window assignment (degree-balanced snake over sorted degrees) ----
    indeg = np.bincount(dst, minlength=n)
    prow = np.empty(n, dtype=np.int64)
    win_all = np.empty(n, dtype=np.int64)
    for c in range(n_cores):
        lo = c * npc
        order = np.argsort(-indeg[lo : lo + npc], kind="stable")
        pos = np.empty(npc, dtype=np.int64)
        pos[order] = np.arange(npc)
        rnd = pos // nwin
        off = pos % nwin
        w = np.where(rnd % 2 == 0, off, nwin - 1 - off)
        plid = (w // 4) * SUP + (w % 4) * WIN + rnd
        prow[lo : lo + npc] = c * padc + plid
        win_all[lo : lo + npc] = w
    p.prow = prow

    # ---- per-core-window tile budgets (uniform across cores) ----
    ecore = dst // npc
    esrc_prow = prow[src]
    can_lo = esrc_prow < IDX_CAP
    can_hi = esrc_prow >= base_hi
    ewin = win_all[dst]

    flo_cw = np.zeros((n_cores, nwin), dtype=np.int64)
    fhi_cw = np.zeros((n_cores, nwin), dtype=np.int64)
    tot_cw = np.zeros((n_cores, nwin), dtype=np.int64)
    np.add.at(tot_cw, (ecore, ewin), 1)
    np.add.at(flo_cw, (ecore[~can_hi], ewin[~can_hi]), 1)
    np.add.at(fhi_cw, (ecore[~can_lo], ewin[~can_lo]), 1)

    t_lo = np.max((flo_cw + 127) // 128, axis=0)
    t_hi = np.max((fhi_cw + 127) // 128, axis=0)
    grow = np.maximum(np.max(tot_cw, axis=0) - (t_lo + t_hi) * 128, 0)
    t_lo = t_lo + (grow + 127) // 128
    t_lo = np.maximum(t_lo, (t_lo + t_hi) == 0)
    p.t_lo, p.t_hi = t_lo, t_hi

    # ---- global tile order: groups of supertiles, class runs within group --
    n_groups = (nsup + GROUP_SUPS - 1) // GROUP_SUPS
    p.n_groups = n_groups
    tiles = []  # (win, cls)
    chunks = []  # (tile_start, ntiles, cls)
    groups = []  # (sup_start, nsups, [chunk idx], (t0, t1))
    for g in range(n_groups):
        s0 = g * GROUP_SUPS
        ns = min(GROUP_SUPS, nsup - s0)
        wlist = range(s0 * 4, (s0 + ns) * 4)
        g_t0 = len(tiles)
        g_chunks = []
        for cls in (0, 1):
            run_t0 = len(tiles)
            for w in wlist:
                tc = int(t_lo[w]) if cls == 0 else int(t_hi[w])
                tiles.extend((w, cls) for _ in range(tc))
            nrun = len(tiles) - run_t0
            t0 = run_t0
            while nrun > 0:
                take = min(CHUNK_MAX, nrun)
                g_chunks.append(len(chunks))
                chunks.append((t0, take, cls))
                t0 += take
                nrun -= take
        groups.append((s0, ns, g_chunks, (g_t0, len(tiles))))
    p.tiles, p.chunks, p.groups = tiles, chunks, groups
    p.tot = len(tiles)

    # queue assignment: greedy least-loaded (by tile count) over queues 1..3
    qload = {q: 0 for q in GATHER_QUEUES}
    p.chunk_queue = []
    for _t0, ntl, _cls in chunks:
        q = min(qload, key=lambda k: qload[k])
        p.chunk_queue.append(q)
        qload[q] += ntl

    # first/last tile of each window (for PSUM start/stop flags)
    tw = np.array([t[0] for t in tiles])
    p.first_of_win = np.zeros(p.tot, dtype=bool)
    p.last_of_win = np.zeros(p.tot, dtype=bool)
    for w in range(nwin):
        ids = np.nonzero(tw == w)[0]
        p.first_of_win[ids.min()] = True
        p.last_of_win[ids.max()] = True
    # first tile index of each (win, cls) run
    t_off = {}
    for t, (w, cls) in enumerate(tiles):
        t_off.setdefault((w, cls), t)

    counts = np.bincount(batch, minlength=n_graphs).astype(np.float32)
    p.counts = counts

    # ---- per-core arrays ----
    norm = dinv[src] * ew * dinv[dst]  # full edge coefficient
    p.per_core = []
    for c in range(n_cores):
        m = ecore == c
        ed = dst[m]
        enorm = norm[m]
        eprow = esrc_prow[m]
        ewin_c = ewin[m]
        eslot = (prow[ed] % SUP) % WIN  # row within window = rnd
        e_can_hi = can_hi[m]
        e_can_lo = can_lo[m]
        ne = len(ed)

        # per-edge class: fill lo up to its target, rest hi
        ecls = np.full(ne, -1, dtype=np.int64)
        ecls[~e_can_hi] = 0
        ecls[~e_can_lo] = 1
        free = ecls == -1
        tot_w = np.bincount(ewin_c, minlength=nwin)
        flo_w = np.bincount(ewin_c[~e_can_hi], minlength=nwin)
        lo_target = np.maximum(flo_w, tot_w - t_hi * 128)
        lo_target = np.minimum(lo_target, t_lo * 128)
        fidx = np.nonzero(free)[0]
        frank = _group_ranks(ewin_c[fidx], nwin)
        to_lo = frank < (lo_target - flo_w)[ewin_c[fidx]]
        ecls[fidx[to_lo]] = 0
        ecls[fidx[~to_lo]] = 1

        # slot position within (win, cls) run
        key = ewin_c * 2 + ecls
        k = _group_ranks(key, nwin * 2)
        t_off_arr = np.zeros((nwin, 2), dtype=np.int64)
        for (wv, cv), tv in t_off.items():
            t_off_arr[wv, cv] = tv
        run0 = t_off_arr[ewin_c, ecls]
        t_of_e = run0 + k // 128
        p_of_e = k % 128

        idx_arr = np.zeros((p.tot, 128), dtype=np.int16)
        s_arr = np.zeros((p.tot, 128, WIN), dtype=np.float16)
        rel = eprow - np.where(ecls == 1, base_hi, 0)
        assert rel.min() >= 0 and rel.max() < IDX_CAP
        idx_arr[t_of_e, p_of_e] = rel.astype(np.int16)
        s_arr[t_of_e, p_of_e, eslot] = enorm.astype(np.float16)

        # wrapped idx layout [16, tot*8], replicated to [128, tot*8]
        idx16 = np.zeros((16, p.tot * 8), dtype=np.int16)
        for ppart in range(128):
            idx16[ppart % 16, np.arange(p.tot) * 8 + ppart // 16] = idx_arr[:, ppart]
        idx128 = np.ascontiguousarray(np.tile(idx16, (8, 1)))

        # S tiles SBUF layout [128 slot, tot*WIN] fp16
        s_sb = np.ascontiguousarray(
            s_arr.transpose(1, 0, 2).reshape(128, p.tot * WIN)
        )
        p.per_core.append(dict(idx128=idx128, s_sb=s_sb))

    # ---- node-indexed arrays (host layouts) ----
    xf = np.asarray(x, dtype=np.float32)
    # layer-1 table: plain x, zero-padded to [npad, HID] fp16
    tab1 = np.zeros((npad, HID), dtype=np.float16)
    tab1[prow, :F_IN] = xf.astype(np.float16)
    p.tab1 = tab1
    # per-core: x2own = x*dinv^2 [128, nsup, F_PAD] f32 and dinv2 [128, nsup]
    p.x2own = []
    p.dinv2 = []
    p.pmat = []
    for c in range(n_cores):
        lo = c * npc
        plid = prow[lo : lo + npc] - c * padc
        xo = np.zeros((128, nsup, F_PAD), dtype=np.float32)
        d2 = np.zeros((128, nsup), dtype=np.float32)
        dv2 = dinv[lo : lo + npc] * dinv[lo : lo + npc]
        xo[plid % 128, plid // 128, :F_IN] = xf[lo : lo + npc] * dv2[:, None]
        d2[plid % 128, plid // 128] = dv2
        p.x2own.append(np.ascontiguousarray(xo))
        p.dinv2.append(np.ascontiguousarray(d2))
        pm = np.zeros((128, nsup * n_graphs), dtype=np.float16)
        pm[plid % 128, (plid // 128) * n_graphs + batch[lo : lo + npc]] = 1.0
        p.pmat.append(np.ascontiguousarray(pm))

    p.identity = np.eye(128, dtype=np.float32)
    return p


def build_weight_arrays(p, W1, b1, W2, b2, W3, b3, Wl, bl):
    """Zero-pad / reshape weights (no arithmetic)."""
    w1p = np.zeros((F_PAD, HID), dtype=np.float32)
    w1p[:F_IN] = np.asarray(W1, dtype=np.float32)
    a = dict(
        w1=w1p,
        w2=np.asarray(W2, dtype=np.float32),
        w3=np.asarray(W3, dtype=np.float32),
        wl=np.asarray(Wl, dtype=np.float32),
        b1=np.asarray(b1, dtype=np.float32).reshape(HID, 1),
        b2=np.asarray(b2, dtype=np.float32).reshape(HID, 1),
        b3=np.asarray(b3, dtype=np.float32).reshape(HID, 1),
        blrep=np.ascontiguousarray(
            np.broadcast_to(np.asarray(bl, dtype=np.float32), (p.n_graphs, N_CLASS))
        ),
        invc=(1.0 / np.maximum(p.counts, 1.0)).reshape(p.n_graphs, 1),
    )
    return a


# ----------------------------------------------------------------------------
# Device program
# ----------------------------------------------------------------------------
def build_program(p, enable_asserts=False):
    import concourse.bass as bass
    import concourse.bacc as bacc
    import concourse.tile as tile
    import concourse.mybir as mybir

    dt = mybir.dt
    f32, f16, i16 = dt.float32, dt.float16, dt.int16
    Alu = mybir.AluOpType
    Act = mybir.ActivationFunctionType
    G = p.n_graphs
    rg = [list(range(p.n_cores))]

    nc = bacc.Bacc(
        "TRN2",
        target_bir_lowering=False,
        debug=False,
        enable_asserts=enable_asserts,
        num_devices=p.n_cores,
        num_swdge_queues=4,
    )

    # ---- DRAM tensors ----
    tab1_d = nc.dram_tensor("tab1", [p.npad, HID], f16, kind="ExternalInput")
    x2own_d = nc.dram_tensor("x2own", [128, p.nsup, F_PAD], f32, kind="ExternalInput")
    dinv2_d = nc.dram_tensor("dinv2", [128, p.nsup], f32, kind="ExternalInput")
    idx_d = nc.dram_tensor("idx", [128, p.tot * 8], i16, kind="ExternalInput")
    s_d = nc.dram_tensor("s_sb", [128, p.tot * WIN], f16, kind="ExternalInput")
    pmat_d = nc.dram_tensor("pmat", [128, p.nsup * G], f16, kind="ExternalInput")
    w1_d = nc.dram_tensor("w1", [F_PAD, HID], f32, kind="ExternalInput")
    w2_d = nc.dram_tensor("w2", [HID, HID], f32, kind="ExternalInput")
    w3_d = nc.dram_tensor("w3", [HID, HID], f32, kind="ExternalInput")
    wl_d = nc.dram_tensor("wl", [HID, N_CLASS], f32, kind="ExternalInput")
    b1_d = nc.dram_tensor("b1", [HID, 1], f32, kind="ExternalInput")
    b2_d = nc.dram_tensor("b2", [HID, 1], f32, kind="ExternalInput")
    b3_d = nc.dram_tensor("b3", [HID, 1], f32, kind="ExternalInput")
    invc_d = nc.dram_tensor("invc", [G, 1], f32, kind="ExternalInput")
    blrep_d = nc.dram_tensor("blrep", [G, N_CLASS], f32, kind="ExternalInput")
    ident_d = nc.dram_tensor("ident", [128, 128], f32, kind="ExternalInput")
    out_d = nc.dram_tensor("out", [G, N_CLASS], f32, kind="ExternalOutput")

    agin_d = nc.dram_tensor("agin", [p.padc, HID], f16, kind="Internal")
    table2_d = nc.dram_tensor(
        "table2", [p.npad, HID], f16, kind="Internal", addr_space="Shared"
    )
    table3_d = nc.dram_tensor(
        "table3", [p.npad, HID], f16, kind="Internal", addr_space="Shared"
    )
    # gathers from the Shared region drain ~30% slower than from normal DRAM;
    # copy each AllGather result into a normal Internal tensor and gather there
    table2n_d = nc.dram_tensor("table2n", [p.npad, HID], f16, kind="Internal")
    table3n_d = nc.dram_tensor("table3n", [p.npad, HID], f16, kind="Internal")
    arin_d = nc.dram_tensor("arin", [128, G], f32, kind="Internal")
    arout_d = nc.dram_tensor(
        "arout", [128, G], f32, kind="Internal", addr_space="Shared"
    )
    warm_in_d = nc.dram_tensor("warm_in", [128, 8], f32, kind="Internal")
    warm_out_d = nc.dram_tensor(
        "warm_out", [128, 64], f32, kind="Internal", addr_space="Shared"
    )

    with tile.TileContext(nc) as tc:
        with (
            tc.tile_pool(name="const", bufs=1) as cpool,
            tc.tile_pool(name="gath", bufs=GATH_BUFS) as gpool,
            tc.tile_pool(name="stage", bufs=3) as stpool,
            tc.tile_pool(name="psagg", bufs=GROUP_SUPS + 1, space="PSUM") as psagg,
            tc.tile_pool(name="psstg", bufs=2, space="PSUM") as psstg,
            tc.tile_pool(name="psacc", bufs=1, space="PSUM") as psacc,
        ):
            # ---- persistent SBUF tiles ----
            ident = cpool.tile([128, 128], f32, tag="ident")
            nc.sync.dma_start(ident[:, :], ident_d[:, :])
            w1 = cpool.tile([F_PAD, HID], f32, tag="w1")
            nc.sync.dma_start(w1[:, :], w1_d[:, :])
            w2 = cpool.tile([HID, HID], f32, tag="w2")
            nc.sync.dma_start(w2[:, :], w2_d[:, :])
            w3 = cpool.tile([HID, HID], f32, tag="w3")
            nc.sync.dma_start(w3[:, :], w3_d[:, :])
            wl = cpool.tile([HID, N_CLASS], f32, tag="wl")
            nc.sync.dma_start(wl[:, :], wl_d[:, :])
            b1 = cpool.tile([HID, 1], f32, tag="b1")
            nc.sync.dma_start(b1[:, :], b1_d[:, :])
            b2 = cpool.tile([HID, 1], f32, tag="b2")
            nc.sync.dma_start(b2[:, :], b2_d[:, :])
            b3 = cpool.tile([HID, 1], f32, tag="b3")
            nc.sync.dma_start(b3[:, :], b3_d[:, :])
            invc = cpool.tile([G, 1], f32, tag="invc")
            nc.sync.dma_start(invc[:, :], invc_d[:, :])
            blrep = cpool.tile([G, N_CLASS], f32, tag="blrep")
            nc.sync.dma_start(blrep[:, :], blrep_d[:, :])
            pmat = cpool.tile([128, p.nsup * G], f16, tag="pmat")
            nc.sync.dma_start(pmat[:, :], pmat_d[:, :])
            idx_sb = cpool.tile([128, p.tot * 8], i16, tag="idx")
            nc.sync.dma_start(idx_sb[:, :], idx_d[:, :])
            s_sb = cpool.tile([128, p.tot * WIN], f16, tag="s_sb")
            nc.sync.dma_start(s_sb[:, :], s_d[:, :])
            dinv2 = cpool.tile([128, p.nsup], f32, tag="dinv2")
            nc.sync.dma_start(dinv2[:, :], dinv2_d[:, :])
            x2own = cpool.tile([128, p.nsup, F_PAD], f32, tag="x2own")
            nc.sync.dma_start(x2own[:, :, :], x2own_d[:, :, :])
            hown2a = cpool.tile([128, p.nsup * HID], f16, tag="hown2a")
            hown2b = cpool.tile([128, p.nsup * HID], f16, tag="hown2b")

            # warm-up collective: absorbs first-collective latency + aligns
            # the cores before the timed layers
            nc.sync.dma_start(warm_in_d[:, :], ident[:, 0:8])
            nc.gpsimd.collective_compute(
                "AllGather",
                Alu.bypass,
                replica_groups=rg,
                ins=[warm_in_d[:, :]],
                outs=[warm_out_d[:, :]],
            )

            # ---- persistent PSUM tiles ----
            pacc = psacc.tile([128, G], f32, tag="pacc")

            # ---- layers ----
            layers = [
                (0, tab1_d, w1, b1, True, (table2_d, table2n_d), None, hown2a),
                (1, table2n_d, w2, b2, True, (table3_d, table3n_d), hown2a, hown2b),
                (2, table3n_d, w3, b3, False, (None, None), hown2b, None),
            ]
            for li, tab_d, w_sb, b_sb, relu, (tab_next, tab_next_n), hin, hout in layers:
                fdim = F_PAD if li == 0 else HID
                lo_view = tab_d[0 : min(p.npad, IDX_CAP), :]
                hi_view = tab_d[p.base_hi : p.npad, :]
                for s0, nsg, chunk_ids, _tr in p.groups:
                    aggs = [
                        psagg.tile([128, HID], f32, tag="agg", name="agg")
                        for _ in range(nsg)
                    ]
                    for ci in chunk_ids:
                        t0, ntl, cls = p.chunks[ci]
                        gt = gpool.tile([128, CHUNK_MAX, HID], f16, tag="gath")
                        view = hi_view if cls == 1 else lo_view
                        nc.gpsimd.dma_gather(
                            gt[:, :ntl, :],
                            view,
                            idx_sb[:, t0 * 8 : (t0 + ntl) * 8],
                            ntl * 128,
                            ntl * 128,
                            HID,
                            elem_step=HID,
                            single_packet=SINGLE_PACKET,
                            queue_num=p.chunk_queue[ci],
                        )
                        for j in range(ntl):
                            t = t0 + j
                            w, _cls = p.tiles[t]
                            sj = (w // 4) - s0
                            pb = (w % 4) * WIN
                            nc.tensor.matmul(
                                aggs[sj][pb : pb + WIN, 0:fdim],
                                s_sb[:, t * WIN : (t + 1) * WIN],
                                gt[:, j, 0:fdim],
                                start=bool(p.first_of_win[t]),
                                stop=bool(p.last_of_win[t]),
                                tile_position=(0, pb),
                                skip_group_check=True,
                            )
                    for sj in range(nsg):
                        s = s0 + sj
                        psum_agg = aggs[sj][:, 0:fdim]
                        stg = psstg.tile([128, 384], f32, tag="stg")
                        uTps = stg[:, 0:128]
                        hTps = stg[:, 128:256]
                        hbps = stg[:, 256:384]
                        u = stpool.tile([128, HID], f32, tag="u")
                        if li == 0:
                            nc.vector.tensor_tensor(
                                u[:, 0:F_PAD],
                                psum_agg,
                                x2own[:, s, :],
                                Alu.add,
                            )
                        else:
                            nc.vector.tensor_tensor(
                                u[:, :],
                                psum_agg,
                                hin[:, s * HID : (s + 1) * HID],
                                Alu.add,
                            )
                        nc.tensor.transpose(
                            uTps[0:fdim, :], u[:, 0:fdim], ident[:, :]
                        )
                        uT = stpool.tile([128, 128], f32, tag="uTs")
                        nc.vector.tensor_copy(uT[0:fdim, :], uTps[0:fdim, :])
                        nc.tensor.matmul(
                            hTps,
                            w_sb[0:fdim, :],
                            uT[0:fdim, :],
                            start=True,
                            stop=True,
                        )
                        hT = stpool.tile([128, 128], f32, tag="hTs")
                        if relu:
                            nc.scalar.activation(
                                hT[:, :],
                                hTps,
                                Act.Relu,
                                bias=b_sb[:, 0:1],
                            )
                        else:
                            nc.vector.tensor_scalar(
                                hT[:, :], hTps, b_sb[:, 0:1], None, Alu.add
                            )
                        nc.tensor.transpose(hbps, hT[:, :], ident[:, :])
                        if li < 2:
                            hf = stpool.tile([128, 128], f16, tag="hf")
                            nc.vector.tensor_copy(hf[:, :], hbps)
                            nc.vector.tensor_scalar(
                                hout[:, s * HID : (s + 1) * HID],
                                hbps,
                                dinv2[:, s : s + 1],
                                None,
                                Alu.mult,
                            )
                            nc.sync.dma_start(
                                agin_d[:, :].rearrange("(t q) f -> q t f", q=128)[
                                    :, s, :
                                ],
                                hf[:, :],
                            )
                        else:
                            h3 = stpool.tile([128, 128], f16, tag="hf")
                            nc.vector.tensor_copy(h3[:, :], hbps)
                            nc.tensor.matmul(
                                pacc[:, 0:G],
                                h3[:, :],
                                pmat[:, s * G : (s + 1) * G],
                                start=(s == 0),
                                stop=(s == p.nsup - 1),
                                skip_group_check=True,
                            )
                if tab_next is not None:
                    nc.gpsimd.collective_compute(
                        "AllGather",
                        Alu.bypass,
                        replica_groups=rg,
                        ins=[agin_d[:, :]],
                        outs=[tab_next[:, :]],
                    )
                    nc.sync.dma_start(tab_next_n[:, :], tab_next[:, :])

            # ---- pooling finalize + classifier ----
            pooledT = stpool.tile([128, G], f32, tag="pool")
            nc.vector.tensor_copy(pooledT[:, :], pacc[:, 0:G])
            nc.sync.dma_start(arin_d[:, :], pooledT[:, :])
            nc.gpsimd.collective_compute(
                "AllReduce",
                Alu.add,
                replica_groups=rg,
                ins=[arin_d[:, :]],
                outs=[arout_d[:, :]],
            )
            pooled2 = stpool.tile([128, G], f32, tag="pool")
            nc.sync.dma_start(pooled2[:, :], arout_d[:, :])
            lgps = psstg.tile([128, 384], f32, tag="stg")
            nc.tensor.matmul(
                lgps[0:G, 0:N_CLASS], pooled2[:, :], wl[:, :], start=True, stop=True
            )
            outt = stpool.tile([G, N_CLASS], f32, tag="out")
            nc.vector.scalar_tensor_tensor(
                outt[:, :],
                lgps[0:G, 0:N_CLASS],
                invc[:, 0:1],
                blrep[:, :],
                Alu.mult,
                Alu.add,
            )
            nc.sync.dma_start(out_d[:, :], outt[:, :])

    nc.compile()
    return nc


def make_in_maps(p, wa):
    maps = []
    for c in range(p.n_cores):
        pc = p.per_core[c]
        maps.append(
            dict(
                tab1=p.tab1,
                x2own=p.x2own[c],
                dinv2=p.dinv2[c],
                idx=pc["idx128"],
                s_sb=pc["s_sb"],
                pmat=p.pmat[c],
                w1=wa["w1"],
                w2=wa["w2"],
                w3=wa["w3"],
                wl=wa["wl"],
                b1=wa["b1"],
                b2=wa["b2"],
                b3=wa["b3"],
                invc=wa["invc"],
                blrep=wa["blrep"],
                ident=p.identity,
            )
        )
    return maps


_CACHE = {}


def kernel(x, edge_index, edge_attr, batch, W1, b1, W2, b2, W3, b3, Wl, bl):
    x = np.asarray(x)
    p = build_plan(x, np.asarray(edge_index), np.asarray(edge_attr), np.asarray(batch))
    wa = build_weight_arrays(p, W1, b1, W2, b2, W3, b3, Wl, bl)
    key = (p.n, p.tot)
    if key not in _CACHE:
        _CACHE[key] = build_program(p)
    nc = _CACHE[key]
    from concourse.bass_utils import run_bass_kernel_spmd

    res = run_bass_kernel_spmd(nc, make_in_maps(p, wa), core_ids=list(range(p.n_cores)))
    return np.asarray(res.results[0]["out"], dtype=np.float32)


# revision 20
# speedup vs baseline: 2.6322x; 1.5750x over previous
"""GCN (3x GCNConv + global mean pool + linear) on 8 Trainium2 NeuronCores.

Strategy (dst-sharded message passing, v2):
  - Nodes sharded n/8 per core; each core's nodes permuted into windows of
    32 (degree-balanced) -> supertiles of 128 (PSUM tiles).
  - Edges partitioned by dst core and packed into (window, class) tiles of
    128 edges; class = which signed-int16-indexable half of the node table
    the src row lives in (dma_gather indices are int16).
  - Normalization dinv = rsqrt(deg+1) is computed on host and folded into
    host-built scatter tiles S [128e, 32d] fp16 (coef = dinv_src*ew*dinv_dst),
    kept SBUF-resident for all three layers. Tables store PLAIN h (fp16).
  - Per layer: dma_gather pulls 256B fp16 rows of the node table from HBM,
    rotating over SWDGE queues 1..3 so three Q7 core-pairs generate DMA
    descriptors concurrently (queue 0 gathers run synchronously on the Pool
    engine; 1..3 retire early and overlap); TensorE computes the
    scatter-add as S^T @ M matmuls accumulated per window in PSUM.
  - Per supertile: u = agg + hown2 (hown2 = h*dinv^2, resident in SBUF),
    PE transpose, f32 GEMM with W, bias(+relu) on ACT, transpose back,
    fp16 table write. Tables distributed with AllGather.
  - Pooling: matmul with host-built P (1[batch==g]) accumulated over
    supertiles -> AllReduce -> final linear on-device -> out [64, 5] f32.
"""

import os
import sys
import numpy as np

for _p in ("/opt/trn_rl_repo", "/root/.axon_site/_ro/trn_rl_repo"):
    if os.path.isdir(_p) and _p not in sys.path:
        sys.path.insert(0, _p)

N_CORES = 8
N_GRAPHS = 64
HID = 128
N_CLASS = 5
F_IN = 7
F_PAD = 8
WIN = 32
SUP = 128
GROUP_SUPS = 3
CHUNK_MAX = 32
GATH_BUFS = 6
SINGLE_PACKET = False
IDX_CAP = 32768
GATHER_QUEUES = (1, 2, 3)


def _group_ranks(keys, n_keys):
    """rank of each element within its key group (keys int array)."""
    nk = len(keys)
    if nk == 0:
        return np.zeros(0, dtype=np.int64)
    order = np.argsort(keys, kind="stable")
    sk = keys[order]
    is_new = np.r_[True, sk[1:] != sk[:-1]]
    gs_idx = np.nonzero(is_new)[0]
    gs = np.repeat(gs_idx, np.diff(np.r_[gs_idx, nk]))
    rank = np.empty(nk, dtype=np.int64)
    rank[order] = np.arange(nk) - gs
    return rank


class Plan:
    pass


def build_plan(x, edge_index, edge_attr, batch, n_cores=N_CORES, n_graphs=N_GRAPHS):
    """Host-side sharding/layout planning + normalization coefficients."""
    p = Plan()
    n = x.shape[0]
    assert n % n_cores == 0
    npc = n // n_cores
    nsup = (npc + SUP - 1) // SUP
    padc = nsup * SUP
    npad = n_cores * padc
    nwin = padc // WIN
    p.n, p.npc, p.nsup, p.padc, p.npad, p.nwin = n, npc, nsup, padc, npad, nwin
    p.n_cores, p.n_graphs = n_cores, n_graphs
    base_hi = max(0, npad - IDX_CAP)
    p.base_hi = base_hi

    src = np.asarray(edge_index[0], dtype=np.int64)
    dst = np.asarray(edge_index[1], dtype=np.int64)
    ew = np.asarray(edge_attr, dtype=np.float32)
    batch = np.asarray(batch, dtype=np.int64)

    # ---- normalization (host): deg = sum of incoming ew + 1 (self loop) ----
    deg = np.bincount(dst, weights=ew.astype(np.float64), minlength=n) + 1.0
    dinv = (1.0 / np.sqrt(deg)).astype(np.float32)
    p.dinv = dinv

    # ---- window assignment: weighted-target greedy balance of edge counts --
    # Target <=512 edges (4 tiles) per window; K coordinated overflow windows
    # absorb each core's excess so the cross-core max stays at the floor.
    import heapq

    indeg = np.bincount(dst, minlength=n)
    core_tot = np.bincount(dst // npc, minlength=n_cores)
    K = int(max(0, (np.max(core_tot) - nwin * 4 * 128 + 127) // 128))
    base = np.full(nwin, 4 * 128.0)
    base[:K] += 128.0
    prow = np.empty(n, dtype=np.int64)
    win_all = np.empty(n, dtype=np.int64)
    for c in range(n_cores):
        lo = c * npc
        deg_c = np.zeros(padc, dtype=np.int64)
        deg_c[:npc] = indeg[lo : lo + npc]  # pad ghost nodes with degree 0
        order = np.argsort(-deg_c, kind="stable")
        tgt = base * (core_tot[c] / base.sum())
        load = np.zeros(nwin)
        wsel = np.empty(padc, dtype=np.int64)
        rnd = np.empty(padc, dtype=np.int64)
        for r in range(WIN):
            block = order[r * nwin : (r + 1) * nwin]  # degrees descending
            worder = np.argsort(load / tgt, kind="stable")  # lightest first
            wsel[block] = worder
            rnd[block] = r
            load[worder] += deg_c[block]
        # repair: node swaps until every window is under its hard cap
        cap = np.full(nwin, 4 * 128.0)
        cap[:K] += 128.0
        for _ in range(2000):
            over = np.nonzero(load > cap)[0]
            if len(over) == 0:
                break
            wh = over[np.argmax((load - cap)[over])]
            excess = load[wh] - cap[wh]
            room = cap - load
            wl = int(np.argmax(room))
            ih = np.nonzero(wsel == wh)[0]
            il = np.nonzero(wsel == wl)[0]
            d = deg_c[ih][:, None] - deg_c[il][None, :]
            valid = (d >= excess) & (d <= room[wl])
            if valid.any():
                dv = np.where(valid, d, np.inf)
                j = np.unravel_index(np.argmin(dv), d.shape)
            else:
                dv = np.where(d <= room[wl], d, -np.inf)
                j = np.unravel_index(np.argmax(dv), d.shape)
                if not np.isfinite(dv[j]) or d[j] <= 0:
                    break
            da, db = ih[j[0]], il[j[1]]
            delta = d[j]
            wsel[da], wsel[db] = wl, wh
            rnd[da], rnd[db] = rnd[db], rnd[da]
            load[wh] -= delta
            load[wl] += delta
        w = wsel[:npc]
        plid = (w // 4) * SUP + (w % 4) * WIN + rnd[:npc]
        prow[lo : lo + npc] = c * padc + plid
        win_all[lo : lo + npc] = w
    p.prow = prow

    # ---- per-core-window tile budgets (uniform across cores) ----
    ecore = dst // npc
    esrc_prow = prow[src]
    can_lo = esrc_prow < IDX_CAP
    can_hi = esrc_prow >= base_hi
    ewin = win_all[dst]

    flo_cw = np.zeros((n_cores, nwin), dtype=np.int64)
    fhi_cw = np.zeros((n_cores, nwin), dtype=np.int64)
    tot_cw = np.zeros((n_cores, nwin), dtype=np.int64)
    np.add.at(tot_cw, (ecore, ewin), 1)
    np.add.at(flo_cw, (ecore[~can_hi], ewin[~can_hi]), 1)
    np.add.at(fhi_cw, (ecore[~can_lo], ewin[~can_lo]), 1)

    t_lo = np.max((flo_cw + 127) // 128, axis=0)
    t_hi = np.max((fhi_cw + 127) // 128, axis=0)
    grow = np.maximum(np.max(tot_cw, axis=0) - (t_lo + t_hi) * 128, 0)
    t_lo = t_lo + (grow + 127) // 128
    t_lo = np.maximum(t_lo, (t_lo + t_hi) == 0)
    p.t_lo, p.t_hi = t_lo, t_hi

    # ---- global tile order: groups of supertiles, class runs within group --
    n_groups = (nsup + GROUP_SUPS - 1) // GROUP_SUPS
    p.n_groups = n_groups
    tiles = []  # (win, cls)
    chunks = []  # (tile_start, ntiles, cls)
    groups = []  # (sup_start, nsups, [chunk idx], (t0, t1))
    for g in range(n_groups):
        s0 = g * GROUP_SUPS
        ns = min(GROUP_SUPS, nsup - s0)
        wlist = range(s0 * 4, (s0 + ns) * 4)
        g_t0 = len(tiles)
        g_chunks = []
        for cls in (0, 1):
            run_t0 = len(tiles)
            for w in wlist:
                tc = int(t_lo[w]) if cls == 0 else int(t_hi[w])
                tiles.extend((w, cls) for _ in range(tc))
            nrun = len(tiles) - run_t0
            t0 = run_t0
            while nrun > 0:
                take = min(CHUNK_MAX, nrun)
                g_chunks.append(len(chunks))
                chunks.append((t0, take, cls))
                t0 += take
                nrun -= take
        groups.append((s0, ns, g_chunks, (g_t0, len(tiles))))
    p.tiles, p.chunks, p.groups = tiles, chunks, groups
    p.tot = len(tiles)

    # queue assignment: greedy least-loaded (by tile count) over queues 1..3
    qload = {q: 0 for q in GATHER_QUEUES}
    p.chunk_queue = []
    for _t0, ntl, _cls in chunks:
        q = min(qload, key=lambda k: qload[k])
        p.chunk_queue.append(q)
        qload[q] += ntl

    # first/last tile of each window (for PSUM start/stop flags)
    tw = np.array([t[0] for t in tiles])
    p.first_of_win = np.zeros(p.tot, dtype=bool)
    p.last_of_win = np.zeros(p.tot, dtype=bool)
    for w in range(nwin):
        ids = np.nonzero(tw == w)[0]
        p.first_of_win[ids.min()] = True
        p.last_of_win[ids.max()] = True
    # first tile index of each (win, cls) run
    t_off = {}
    for t, (w, cls) in enumerate(tiles):
        t_off.setdefault((w, cls), t)

    counts = np.bincount(batch, minlength=n_graphs).astype(np.float32)
    p.counts = counts

    # ---- per-core arrays ----
    norm = dinv[src] * ew * dinv[dst]  # full edge coefficient
    p.per_core = []
    for c in range(n_cores):
        m = ecore == c
        ed = dst[m]
        enorm = norm[m]
        eprow = esrc_prow[m]
        ewin_c = ewin[m]
        eslot = (prow[ed] % SUP) % WIN  # row within window = rnd
        e_can_hi = can_hi[m]
        e_can_lo = can_lo[m]
        ne = len(ed)

        # per-edge class: fill lo up to its target, rest hi
        ecls = np.full(ne, -1, dtype=np.int64)
        ecls[~e_can_hi] = 0
        ecls[~e_can_lo] = 1
        free = ecls == -1
        tot_w = np.bincount(ewin_c, minlength=nwin)
        flo_w = np.bincount(ewin_c[~e_can_hi], minlength=nwin)
        lo_target = np.maximum(flo_w, tot_w - t_hi * 128)
        lo_target = np.minimum(lo_target, t_lo * 128)
        fidx = np.nonzero(free)[0]
        frank = _group_ranks(ewin_c[fidx], nwin)
        to_lo = frank < (lo_target - flo_w)[ewin_c[fidx]]
        ecls[fidx[to_lo]] = 0
        ecls[fidx[~to_lo]] = 1

        # slot position within (win, cls) run
        key = ewin_c * 2 + ecls
        k = _group_ranks(key, nwin * 2)
        t_off_arr = np.zeros((nwin, 2), dtype=np.int64)
        for (wv, cv), tv in t_off.items():
            t_off_arr[wv, cv] = tv
        run0 = t_off_arr[ewin_c, ecls]
        t_of_e = run0 + k // 128
        p_of_e = k % 128

        idx_arr = np.zeros((p.tot, 128), dtype=np.int16)
        s_arr = np.zeros((p.tot, 128, WIN), dtype=np.float16)
        rel = eprow - np.where(ecls == 1, base_hi, 0)
        assert rel.min() >= 0 and rel.max() < IDX_CAP
        idx_arr[t_of_e, p_of_e] = rel.astype(np.int16)
        s_arr[t_of_e, p_of_e, eslot] = enorm.astype(np.float16)

        # wrapped idx layout [16, tot*8], replicated to [128, tot*8]
        idx16 = np.zeros((16, p.tot * 8), dtype=np.int16)
        for ppart in range(128):
            idx16[ppart % 16, np.arange(p.tot) * 8 + ppart // 16] = idx_arr[:, ppart]
        idx128 = np.ascontiguousarray(np.tile(idx16, (8, 1)))

        # S tiles SBUF layout [128 slot, tot*WIN] fp16
        s_sb = np.ascontiguousarray(
            s_arr.transpose(1, 0, 2).reshape(128, p.tot * WIN)
        )
        p.per_core.append(dict(idx128=idx128, s_sb=s_sb))

    # ---- node-indexed arrays (host layouts) ----
    xf = np.asarray(x, dtype=np.float32)
    # layer-1 table: plain x, zero-padded to [npad, HID] fp16
    tab1 = np.zeros((npad, HID), dtype=np.float16)
    tab1[prow, :F_IN] = xf.astype(np.float16)
    p.tab1 = tab1
    # per-core: x2own = x*dinv^2 [128, nsup, F_PAD] f32 and dinv2 [128, nsup]
    p.x2own = []
    p.dinv2 = []
    p.pmat = []
    for c in range(n_cores):
        lo = c * npc
        plid = prow[lo : lo + npc] - c * padc
        xo = np.zeros((128, nsup, F_PAD), dtype=np.float32)
        d2 = np.zeros((128, nsup), dtype=np.float32)
        dv2 = dinv[lo : lo + npc] * dinv[lo : lo + npc]
        xo[plid % 128, plid // 128, :F_IN] = xf[lo : lo + npc] * dv2[:, None]
        d2[plid % 128, plid // 128] = dv2
        p.x2own.append(np.ascontiguousarray(xo))
        p.dinv2.append(np.ascontiguousarray(d2))
        pm = np.zeros((128, nsup * n_graphs), dtype=np.float16)
        pm[plid % 128, (plid // 128) * n_graphs + batch[lo : lo + npc]] = 1.0
        p.pmat.append(np.ascontiguousarray(pm))

    p.identity = np.eye(128, dtype=np.float32)
    return p


def build_weight_arrays(p, W1, b1, W2, b2, W3, b3, Wl, bl):
    """Zero-pad / reshape weights (no arithmetic)."""
    w1p = np.zeros((F_PAD, HID), dtype=np.float32)
    w1p[:F_IN] = np.asarray(W1, dtype=np.float32)
    a = dict(
        w1=w1p,
        w2=np.asarray(W2, dtype=np.float32),
        w3=np.asarray(W3, dtype=np.float32),
        wl=np.asarray(Wl, dtype=np.float32),
        b1=np.asarray(b1, dtype=np.float32).reshape(HID, 1),
        b2=np.asarray(b2, dtype=np.float32).reshape(HID, 1),
        b3=np.asarray(b3, dtype=np.float32).reshape(HID, 1),
        blrep=np.ascontiguousarray(
            np.broadcast_to(np.asarray(bl, dtype=np.float32), (p.n_graphs, N_CLASS))
        ),
        invc=(1.0 / np.maximum(p.counts, 1.0)).reshape(p.n_graphs, 1),
    )
    return a


# ----------------------------------------------------------------------------
# Device program
# ----------------------------------------------------------------------------
def build_program(p, enable_asserts=False):
    import concourse.bass as bass
    import concourse.bacc as bacc
    import concourse.tile as tile
    import concourse.mybir as mybir

    dt = mybir.dt
    f32, f16, i16 = dt.float32, dt.float16, dt.int16
    Alu = mybir.AluOpType
    Act = mybir.ActivationFunctionType
    G = p.n_graphs
    rg = [list(range(p.n_cores))]

    nc = bacc.Bacc(
        "TRN2",
        target_bir_lowering=False,
        debug=False,
        enable_asserts=enable_asserts,
        num_devices=p.n_cores,
        num_swdge_queues=4,
    )

    # ---- DRAM tensors ----
    tab1_d = nc.dram_tensor("tab1", [p.npad, HID], f16, kind="ExternalInput")
    x2own_d = nc.dram_tensor("x2own", [128, p.nsup, F_PAD], f32, kind="ExternalInput")
    dinv2_d = nc.dram_tensor("dinv2", [128, p.nsup], f32, kind="ExternalInput")
    idx_d = nc.dram_tensor("idx", [128, p.tot * 8], i16, kind="ExternalInput")
    s_d = nc.dram_tensor("s_sb", [128, p.tot * WIN], f16, kind="ExternalInput")
    pmat_d = nc.dram_tensor("pmat", [128, p.nsup * G], f16, kind="ExternalInput")
    w1_d = nc.dram_tensor("w1", [F_PAD, HID], f32, kind="ExternalInput")
    w2_d = nc.dram_tensor("w2", [HID, HID], f32, kind="ExternalInput")
    w3_d = nc.dram_tensor("w3", [HID, HID], f32, kind="ExternalInput")
    wl_d = nc.dram_tensor("wl", [HID, N_CLASS], f32, kind="ExternalInput")
    b1_d = nc.dram_tensor("b1", [HID, 1], f32, kind="ExternalInput")
    b2_d = nc.dram_tensor("b2", [HID, 1], f32, kind="ExternalInput")
    b3_d = nc.dram_tensor("b3", [HID, 1], f32, kind="ExternalInput")
    invc_d = nc.dram_tensor("invc", [G, 1], f32, kind="ExternalInput")
    blrep_d = nc.dram_tensor("blrep", [G, N_CLASS], f32, kind="ExternalInput")
    ident_d = nc.dram_tensor("ident", [128, 128], f32, kind="ExternalInput")
    out_d = nc.dram_tensor("out", [G, N_CLASS], f32, kind="ExternalOutput")

    agin_d = nc.dram_tensor("agin", [p.padc, HID], f16, kind="Internal")
    table2_d = nc.dram_tensor(
        "table2", [p.npad, HID], f16, kind="Internal", addr_space="Shared"
    )
    table3_d = nc.dram_tensor(
        "table3", [p.npad, HID], f16, kind="Internal", addr_space="Shared"
    )
    # gathers from the Shared region (and from some ExternalInput placements)
    # drain ~35% slower than from compiler-placed Internal DRAM; gather only
    # from Internal copies of each table
    table2n_d = nc.dram_tensor("table2n", [p.npad, HID], f16, kind="Internal")
    table3n_d = nc.dram_tensor("table3n", [p.npad, HID], f16, kind="Internal")
    tab1n_d = nc.dram_tensor("tab1n", [p.npad, HID], f16, kind="Internal")
    arin_d = nc.dram_tensor("arin", [128, G], f32, kind="Internal")
    arout_d = nc.dram_tensor(
        "arout", [128, G], f32, kind="Internal", addr_space="Shared"
    )
    warm_in_d = nc.dram_tensor("warm_in", [128, 8], f32, kind="Internal")
    warm_out_d = nc.dram_tensor(
        "warm_out", [128, 64], f32, kind="Internal", addr_space="Shared"
    )

    with tile.TileContext(nc) as tc:
        with (
            tc.tile_pool(name="const", bufs=1) as cpool,
            tc.tile_pool(name="gath", bufs=GATH_BUFS) as gpool,
            tc.tile_pool(name="stage", bufs=3) as stpool,
            tc.tile_pool(name="psagg", bufs=GROUP_SUPS + 1, space="PSUM") as psagg,
            tc.tile_pool(name="psstg", bufs=2, space="PSUM") as psstg,
            tc.tile_pool(name="psacc", bufs=1, space="PSUM") as psacc,
        ):
            # ---- persistent SBUF tiles ----
            nc.sync.dma_start(tab1n_d[:, :], tab1_d[:, :])
            ident = cpool.tile([128, 128], f32, tag="ident")
            nc.sync.dma_start(ident[:, :], ident_d[:, :])
            w1 = cpool.tile([F_PAD, HID], f32, tag="w1")
            nc.sync.dma_start(w1[:, :], w1_d[:, :])
            w2 = cpool.tile([HID, HID], f32, tag="w2")
            nc.sync.dma_start(w2[:, :], w2_d[:, :])
            w3 = cpool.tile([HID, HID], f32, tag="w3")
            nc.sync.dma_start(w3[:, :], w3_d[:, :])
            wl = cpool.tile([HID, N_CLASS], f32, tag="wl")
            nc.sync.dma_start(wl[:, :], wl_d[:, :])
            b1 = cpool.tile([HID, 1], f32, tag="b1")
            nc.sync.dma_start(b1[:, :], b1_d[:, :])
            b2 = cpool.tile([HID, 1], f32, tag="b2")
            nc.sync.dma_start(b2[:, :], b2_d[:, :])
            b3 = cpool.tile([HID, 1], f32, tag="b3")
            nc.sync.dma_start(b3[:, :], b3_d[:, :])
            invc = cpool.tile([G, 1], f32, tag="invc")
            nc.sync.dma_start(invc[:, :], invc_d[:, :])
            blrep = cpool.tile([G, N_CLASS], f32, tag="blrep")
            nc.sync.dma_start(blrep[:, :], blrep_d[:, :])
            pmat = cpool.tile([128, p.nsup * G], f16, tag="pmat")
            nc.sync.dma_start(pmat[:, :], pmat_d[:, :])
            idx_sb = cpool.tile([128, p.tot * 8], i16, tag="idx")
            nc.sync.dma_start(idx_sb[:, :], idx_d[:, :])
            s_sb = cpool.tile([128, p.tot * WIN], f16, tag="s_sb")
            nc.sync.dma_start(s_sb[:, :], s_d[:, :])
            dinv2 = cpool.tile([128, p.nsup], f32, tag="dinv2")
            nc.sync.dma_start(dinv2[:, :], dinv2_d[:, :])
            x2own = cpool.tile([128, p.nsup, F_PAD], f32, tag="x2own")
            nc.sync.dma_start(x2own[:, :, :], x2own_d[:, :, :])
            hown2a = cpool.tile([128, p.nsup * HID], f16, tag="hown2a")
            hown2b = cpool.tile([128, p.nsup * HID], f16, tag="hown2b")

            # warm-up collective: absorbs first-collective latency + aligns
            # the cores before the timed layers
            nc.sync.dma_start(warm_in_d[:, :], ident[:, 0:8])
            nc.gpsimd.collective_compute(
                "AllGather",
                Alu.bypass,
                replica_groups=rg,
                ins=[warm_in_d[:, :]],
                outs=[warm_out_d[:, :]],
            )

            # ---- persistent PSUM tiles ----
            pacc = psacc.tile([128, G], f32, tag="pacc")

            # ---- layers ----
            layers = [
                (0, tab1n_d, w1, b1, True, (table2_d, table2n_d), None, hown2a),
                (1, table2n_d, w2, b2, True, (table3_d, table3n_d), hown2a, hown2b),
                (2, table3n_d, w3, b3, False, (None, None), hown2b, None),
            ]
            for li, tab_d, w_sb, b_sb, relu, (tab_next, tab_next_n), hin, hout in layers:
                fdim = F_PAD if li == 0 else HID
                lo_view = tab_d[0 : min(p.npad, IDX_CAP), :]
                hi_view = tab_d[p.base_hi : p.npad, :]
                for s0, nsg, chunk_ids, _tr in p.groups:
                    aggs = [
                        psagg.tile([128, HID], f32, tag="agg", name="agg")
                        for _ in range(nsg)
                    ]
                    for ci in chunk_ids:
                        t0, ntl, cls = p.chunks[ci]
                        gt = gpool.tile([128, CHUNK_MAX, HID], f16, tag="gath")
                        view = hi_view if cls == 1 else lo_view
                        nc.gpsimd.dma_gather(
                            gt[:, :ntl, :],
                            view,
                            idx_sb[:, t0 * 8 : (t0 + ntl) * 8],
                            ntl * 128,
                            ntl * 128,
                            HID,
                            elem_step=HID,
                            single_packet=SINGLE_PACKET,
                            queue_num=p.chunk_queue[ci],
                        )
                        for j in range(ntl):
                            t = t0 + j
                            w, _cls = p.tiles[t]
                            sj = (w // 4) - s0
                            pb = (w % 4) * WIN
                            nc.tensor.matmul(
                                aggs[sj][pb : pb + WIN, 0:fdim],
                                s_sb[:, t * WIN : (t + 1) * WIN],
                                gt[:, j, 0:fdim],
                                start=bool(p.first_of_win[t]),
                                stop=bool(p.last_of_win[t]),
                                tile_position=(0, pb),
                                skip_group_check=True,
                            )
                    for sj in range(nsg):
                        s = s0 + sj
                        psum_agg = aggs[sj][:, 0:fdim]
                        stg = psstg.tile([128, 384], f32, tag="stg")
                        uTps = stg[:, 0:128]
                        hTps = stg[:, 128:256]
                        hbps = stg[:, 256:384]
                        u = stpool.tile([128, HID], f32, tag="u")
                        if li == 0:
                            nc.vector.tensor_tensor(
                                u[:, 0:F_PAD],
                                psum_agg,
                                x2own[:, s, :],
                                Alu.add,
                            )
                        else:
                            nc.vector.tensor_tensor(
                                u[:, :],
                                psum_agg,
                                hin[:, s * HID : (s + 1) * HID],
                                Alu.add,
                            )
                        nc.tensor.transpose(
                            uTps[0:fdim, :], u[:, 0:fdim], ident[:, :]
                        )
                        uT = stpool.tile([128, 128], f32, tag="uTs")
                        nc.vector.tensor_copy(uT[0:fdim, :], uTps[0:fdim, :])
                        nc.tensor.matmul(
                            hTps,
                            w_sb[0:fdim, :],
                            uT[0:fdim, :],
                            start=True,
                            stop=True,
                        )
                        hT = stpool.tile([128, 128], f32, tag="hTs")
                        if relu:
                            nc.scalar.activation(
                                hT[:, :],
                                hTps,
                                Act.Relu,
                                bias=b_sb[:, 0:1],
                            )
                        else:
                            nc.vector.tensor_scalar(
                                hT[:, :], hTps, b_sb[:, 0:1], None, Alu.add
                            )
                        nc.tensor.transpose(hbps, hT[:, :], ident[:, :])
                        if li < 2:
                            hf = stpool.tile([128, 128], f16, tag="hf")
                            nc.vector.tensor_copy(hf[:, :], hbps)
                            nc.vector.tensor_scalar(
                                hout[:, s * HID : (s + 1) * HID],
                                hbps,
                                dinv2[:, s : s + 1],
                                None,
                                Alu.mult,
                            )
                            nc.sync.dma_start(
                                agin_d[:, :].rearrange("(t q) f -> q t f", q=128)[
                                    :, s, :
                                ],
                                hf[:, :],
                            )
                        else:
                            h3 = stpool.tile([128, 128], f16, tag="hf")
                            nc.vector.tensor_copy(h3[:, :], hbps)
                            nc.tensor.matmul(
                                pacc[:, 0:G],
                                h3[:, :],
                                pmat[:, s * G : (s + 1) * G],
                                start=(s == 0),
                                stop=(s == p.nsup - 1),
                                skip_group_check=True,
                            )
                if tab_next is not None:
                    nc.gpsimd.collective_compute(
                        "AllGather",
                        Alu.bypass,
                        replica_groups=rg,
                        ins=[agin_d[:, :]],
                        outs=[tab_next[:, :]],
                    )
                    nc.sync.dma_start(tab_next_n[:, :], tab_next[:, :])

            # ---- pooling finalize + classifier ----
            pooledT = stpool.tile([128, G], f32, tag="pool")
            nc.vector.tensor_copy(pooledT[:, :], pacc[:, 0:G])
            nc.sync.dma_start(arin_d[:, :], pooledT[:, :])
            nc.gpsimd.collective_compute(
                "AllReduce",
                Alu.add,
                replica_groups=rg,
                ins=[arin_d[:, :]],
                outs=[arout_d[:, :]],
            )
            pooled2 = stpool.tile([128, G], f32, tag="pool")
            nc.sync.dma_start(pooled2[:, :], arout_d[:, :])
            lgps = psstg.tile([128, 384], f32, tag="stg")
            nc.tensor.matmul(
                lgps[0:G, 0:N_CLASS], pooled2[:, :], wl[:, :], start=True, stop=True
            )
            outt = stpool.tile([G, N_CLASS], f32, tag="out")
            nc.vector.scalar_tensor_tensor(
                outt[:, :],
                lgps[0:G, 0:N_CLASS],
                invc[:, 0:1],
                blrep[:, :],
                Alu.mult,
                Alu.add,
            )
            nc.sync.dma_start(out_d[:, :], outt[:, :])

    nc.compile()
    return nc


def make_in_maps(p, wa):
    maps = []
    for c in range(p.n_cores):
        pc = p.per_core[c]
        maps.append(
            dict(
                tab1=p.tab1,
                x2own=p.x2own[c],
                dinv2=p.dinv2[c],
                idx=pc["idx128"],
                s_sb=pc["s_sb"],
                pmat=p.pmat[c],
                w1=wa["w1"],
                w2=wa["w2"],
                w3=wa["w3"],
                wl=wa["wl"],
                b1=wa["b1"],
                b2=wa["b2"],
                b3=wa["b3"],
                invc=wa["invc"],
                blrep=wa["blrep"],
                ident=p.identity,
            )
        )
    return maps


_CACHE = {}


def kernel(x, edge_index, edge_attr, batch, W1, b1, W2, b2, W3, b3, Wl, bl):
    x = np.asarray(x)
    p = build_plan(x, np.asarray(edge_index), np.asarray(edge_attr), np.asarray(batch))
    wa = build_weight_arrays(p, W1, b1, W2, b2, W3, b3, Wl, bl)
    key = (p.n, p.tot)
    if key not in _CACHE:
        _CACHE[key] = build_program(p)
    nc = _CACHE[key]
    from concourse.bass_utils import run_bass_kernel_spmd

    res = run_bass_kernel_spmd(nc, make_in_maps(p, wa), core_ids=list(range(p.n_cores)))
    return np.asarray(res.results[0]["out"], dtype=np.float32)
